# revision 41
# baseline (speedup 1.0000x reference)
"""DeltaNet Trainium2 kernel — 8-core SPMD, one (batch, head) pair per core.

Slab-interleaved pipeline: for each 512-token slab, emit projections+conv+silu
(PE-dense), then the token-scalar rows (beta/l2 via Act/DVE), then the two
chunk-pair UT-transform ladders, then the previous slab's serial state chains
and o_proj trailing work.  bf16 datapath with fp32 PSUM accumulation; the
chunked delta rule precomputes Tt = ((I+X)(I+X^2)(I+X^4))^T per chunk off the
serial chain, with both chunks of a pair sharing PSUM/SBUF pair tiles (all
ladder products are single-shot matmuls).  Host folds norm_w into Wo and sums
the 4 per-head partial o_proj outputs per batch.
"""

import os
import sys
from contextlib import ExitStack

import ml_dtypes
import numpy as np

for _p in ("/opt/trn_rl_repo", "/root/.axon_site/_ro/trn_rl_repo"):
    if os.path.isdir(_p) and _p not in sys.path:
        sys.path.insert(0, _p)

import concourse.bass as bass  # noqa: E402
import concourse.tile as tile  # noqa: E402
from concourse import bacc, mybir  # noqa: E402
from concourse.bass_utils import run_bass_kernel_spmd  # noqa: E402

F32 = mybir.dt.float32
F32R = mybir.dt.float32r
BF16 = mybir.dt.bfloat16
AF = mybir.ActivationFunctionType
OP = mybir.AluOpType

HID = 1024
D = 256
C = 128
KT = HID // 128
NH = 4
B = 2
S_FULL = 2048


def build_nc(nchunk=S_FULL // C, dbg=False):
    S = nchunk * C
    scs = 512 if S >= 512 else S
    nsc = S // scs
    cps = scs // C  # chunks per slab (4)
    nc = bacc.Bacc("TRN2", target_bir_lowering=False, debug=False)

    xt_d = nc.dram_tensor("xt", [HID, S], BF16, kind="ExternalInput")
    wq_d = nc.dram_tensor("wq", [HID, D], BF16, kind="ExternalInput")
    wk_d = nc.dram_tensor("wk", [HID, D], BF16, kind="ExternalInput")
    wv_d = nc.dram_tensor("wv", [HID, D], BF16, kind="ExternalInput")
    wb_d = nc.dram_tensor("wb", [HID, 1], BF16, kind="ExternalInput")
    wo_d = nc.dram_tensor("wo", [D, HID], BF16, kind="ExternalInput")
    cdq_d = nc.dram_tensor("cdq", [128, 8 * 128], BF16, kind="ExternalInput")
    cdk_d = nc.dram_tensor("cdk", [128, 8 * 128], BF16, kind="ExternalInput")
    cdv_d = nc.dram_tensor("cdv", [128, 8 * 128], BF16, kind="ExternalInput")
    identb_d = nc.dram_tensor("identb", [128, 128], BF16, kind="ExternalInput")
    identf_d = nc.dram_tensor("identf", [128, 128], F32R, kind="ExternalInput")
    identb2_d = nc.dram_tensor("identb2", [128, 256], BF16, kind="ExternalInput")
    onescol_d = nc.dram_tensor("onescol", [128, 1], BF16, kind="ExternalInput")
    mlow_d = nc.dram_tensor("mlow", [128, 128], F32, kind="ExternalInput")
    mup_d = nc.dram_tensor("mup", [128, 128], F32, kind="ExternalInput")
    out_d = nc.dram_tensor("out", [S, HID], F32, kind="ExternalOutput")

    with tile.TileContext(nc) as tc, ExitStack() as ctx:
        # ---------------- pools ----------------
        pconst = ctx.enter_context(tc.tile_pool(name="pconst", bufs=1))
        pplane = ctx.enter_context(tc.tile_pool(name="pplane", bufs=1))
        pw = ctx.enter_context(tc.tile_pool(name="pw", bufs=1))
        pxt = ctx.enter_context(tc.tile_pool(name="pxt", bufs=1))
        praw = ctx.enter_context(tc.tile_pool(name="praw", bufs=1))
        prow = ctx.enter_context(tc.tile_pool(name="prow", bufs=2))
        pcsq = ctx.enter_context(tc.tile_pool(name="pcsq", bufs=3))
        pS = ctx.enter_context(tc.tile_pool(name="pS", bufs=2))
        pcs = ctx.enter_context(tc.tile_pool(name="pcs", bufs=2))
        pcm = ctx.enter_context(tc.tile_pool(name="pcm", bufs=2))
        pwin = ctx.enter_context(tc.tile_pool(name="pwin", bufs=4))
        ptok = ctx.enter_context(tc.tile_pool(name="ptok", bufs=nchunk))
        pout = ctx.enter_context(tc.tile_pool(name="pout", bufs=2))
        # PSUM: 8 banks total (2 state + 1 chain + 1 pso + 4 shared)
        ppS = ctx.enter_context(tc.tile_pool(name="ppS", bufs=1, space="PSUM"))
        ppw = ctx.enter_context(tc.tile_pool(name="ppw", bufs=1, space="PSUM"))
        ppo = ctx.enter_context(tc.tile_pool(name="ppo", bufs=1, space="PSUM"))
        ppt = ctx.enter_context(tc.tile_pool(name="ppt", bufs=4, space="PSUM"))
        ppb = ppt

        identb = pconst.tile([128, 128], BF16)
        identb2 = pconst.tile([128, 256], BF16)
        identf = pconst.tile([128, 128], F32R)
        onescol = pconst.tile([128, 1], BF16)
        rows4 = pconst.tile([4, S], F32R)  # 0=bk, 1=nbk2, 2=aq, 3=aq2
        mlow = pconst.tile([128, 128], F32)
        mup = pconst.tile([128, 128], F32)
        eps6 = pconst.tile([128, 1], F32)
        eps5 = pconst.tile([128, 1], F32)
        nc.vector.memset(eps6, 1e-6)
        nc.vector.memset(eps5, 1e-5)

        wo_sb = pw.tile([128, 2, HID], BF16)
        wb_sb = pw.tile([128, KT, 1], BF16)

        # planes: kq{dt} holds k in [:,0,:] and q in [:,1,:]; v separate
        kq0 = pplane.tile([128, 2, S], BF16, name="kq0")
        kq1 = pplane.tile([128, 2, S], BF16, name="kq1")
        v0 = pplane.tile([128, S], BF16, name="v0")
        v1 = pplane.tile([128, S], BF16, name="v1")

        xt_sb = pxt.tile([128, KT, S], BF16)

        # ---------------- input DMAs (slab-major xt for early start) -------
        nc.sync.dma_start(
            out=wb_sb, in_=wb_d.ap().rearrange("(k p) o -> p k o", p=128)
        )
        wd_srcs = {"q": wq_d, "k": wk_d, "v": wv_d}
        w_sbs = {}
        for t in ("k", "q", "v"):
            w_sbs[t] = pw.tile([128, KT, D], BF16, name=f"w_{t}")
            nc.sync.dma_start(
                out=w_sbs[t],
                in_=wd_srcs[t].ap().rearrange("(k p) d -> p k d", p=128),
            )
        xt_src = xt_d.ap().rearrange("(k p) s -> p k s", p=128)
        for sc in range(nsc):
            sl = slice(sc * scs, (sc + 1) * scs)
            for kk in range(KT):
                nc.sync.dma_start(out=xt_sb[:, kk, sl], in_=xt_src[:, kk, sl])
        diags = {}
        for t, cd_d in (("k", cdk_d), ("q", cdq_d), ("v", cdv_d)):
            diags[t] = pw.tile([128, 8 * 128], BF16, name=f"diag_{t}")
            nc.sync.dma_start(out=diags[t], in_=cd_d.ap())
        nc.gpsimd.dma_start(out=identb, in_=identb_d.ap())
        nc.gpsimd.dma_start(out=identb2, in_=identb2_d.ap())
        nc.gpsimd.dma_start(out=identf, in_=identf_d.ap())
        nc.gpsimd.dma_start(out=onescol, in_=onescol_d.ap())
        nc.gpsimd.dma_start(out=mlow, in_=mlow_d.ap())
        nc.gpsimd.dma_start(out=mup, in_=mup_d.ap())
        nc.sync.dma_start(
            out=wo_sb, in_=wo_d.ap().rearrange("(t p) h -> p t h", p=128)
        )

        raws = {}
        for t in ("k", "q", "v"):
            for dt_ in range(2):
                raws[(t, dt_)] = praw.tile(
                    [128, S + 8], BF16, name=f"raw_{t}{dt_}"
                )
                nc.gpsimd.memset(raws[(t, dt_)][:, 0:8], 0.0)

        copy_flip = [0]

        # ---------------- phase B slab: proj + conv + silu ----------------
        def emit_B_slab(sc):
            base = sc * scs
            for t in ("k", "q", "v"):
                diag = diags[t]
                for dt_ in range(2):
                    raw = raws[(t, dt_)]
                    ps = ppb.tile([128, scs], F32, tag="ps", name="psraw")
                    for kk in range(KT):
                        nc.tensor.matmul(
                            ps,
                            w_sbs[t][:, kk, dt_ * 128 : (dt_ + 1) * 128],
                            xt_sb[:, kk, base : base + scs],
                            start=(kk == 0),
                            stop=(kk == KT - 1),
                        )
                    dst = raw[:, 8 + base : 8 + base + scs]
                    if copy_flip[0] % 2 == 0:
                        nc.vector.tensor_copy(dst, ps)
                    else:
                        nc.scalar.activation(out=dst, in_=ps, func=AF.Copy)
                    copy_flip[0] += 1
                    if t == "v":
                        pdst = (v0, v1)[dt_][:, base : base + scs]
                    else:
                        pdst = (kq0, kq1)[dt_][
                            :, 0 if t == "k" else 1, base : base + scs
                        ]
                    psc = ppb.tile([128, scs], F32, tag="ps", name="psconv")
                    for j in (3, 2, 1, 0):
                        sh = 3 - j
                        dslc = diag[
                            :, (j * 2 + dt_) * 128 : (j * 2 + dt_ + 1) * 128
                        ]
                        nc.tensor.matmul(
                            psc,
                            dslc,
                            raw[:, 8 + base - sh : 8 + base + scs - sh],
                            start=(j == 3),
                            stop=(j == 0),
                        )
                    nc.scalar.activation(out=pdst, in_=psc, func=AF.Silu)

        # ---------------- token-scalar rows for one slab ----------------
        def emit_rows(sc):
            sl = slice(sc * scs, (sc + 1) * scs)
            psb = ppb.tile([1, scs], F32, tag="ps", name="psb")
            for kk in range(KT):
                nc.tensor.matmul(
                    psb, wb_sb[:, kk, :], xt_sb[:, kk, sl],
                    start=(kk == 0), stop=(kk == KT - 1),
                )
            beta_row = prow.tile([1, scs], F32, tag="beta", name="beta_row")
            nc.scalar.activation(out=beta_row, in_=psb, func=AF.Sigmoid)
            for t in ("k", "q"):
                ti = 0 if t == "k" else 1
                psl = ppb.tile([1, scs], F32, tag="ps", name="psl")
                for dt_, kqp in ((0, kq0), (1, kq1)):
                    sq = pcsq.tile([128, scs], BF16, tag="sq", name="sq")
                    nc.vector.tensor_mul(sq, kqp[:, ti, sl], kqp[:, ti, sl])
                    nc.tensor.matmul(
                        psl, onescol, sq, start=(dt_ == 0), stop=(dt_ == 1)
                    )
                srow = prow.tile([1, scs], F32, tag="srow", name="srow")
                nc.scalar.activation(
                    out=srow, in_=psl, func=AF.Sqrt, bias=eps6[0:1, :]
                )
                with nc.allow_low_precision(reason="f32r tok rows"):
                    if t == "k":
                        ak = prow.tile([1, scs], F32, tag="ak", name="ak")
                        bk = prow.tile([1, scs], F32R, tag="bk", name="bk")
                        nb = prow.tile([1, scs], F32R, tag="nb", name="nb")
                        nc.vector.reciprocal(out=ak, in_=srow)
                        nc.vector.tensor_mul(bk, beta_row, ak)
                        nc.vector.scalar_tensor_tensor(
                            out=nb, in0=bk, scalar=-1.0, in1=ak,
                            op0=OP.mult, op1=OP.mult,
                        )
                        nc.sync.dma_start(out=rows4[0:1, sl], in_=bk)
                        nc.sync.dma_start(out=rows4[1:2, sl], in_=nb)
                    else:
                        aqr = prow.tile([1, scs], F32R, tag="aqr", name="aqr")
                        aq2 = prow.tile([1, scs], F32R, tag="aq2", name="aq2")
                        nc.vector.reciprocal(out=aqr, in_=srow)
                        nc.vector.scalar_tensor_tensor(
                            out=aq2, in0=aqr, scalar=1.0 / D, in1=aqr,
                            op0=OP.mult, op1=OP.mult,
                        )
                        nc.sync.dma_start(out=rows4[2:3, sl], in_=aqr)
                        nc.sync.dma_start(out=rows4[3:4, sl], in_=aq2)

        psS0 = ppS.tile([128, 256], F32, tag="psS0", name="psS0")
        psS1 = ppS.tile([128, 256], F32, tag="psS1", name="psS1")

        state = {}

        # ---------------- pass 1: chunk-pair UT-transform ladder ----------
        def emit_pass1_pair(pi):
            ca, cb = 2 * pi, 2 * pi + 1
            chs = [slice(ca * C, (ca + 1) * C), slice(cb * C, (cb + 1) * C)]

            psR = ppt.tile([128, 8], F32R, tag="ps", name="psR")
            nc.tensor.transpose(psR[:, 0:4], rows4[0:4, chs[0]], identf[0:4, 0:4])
            nc.tensor.transpose(psR[:, 4:8], rows4[0:4, chs[1]], identf[0:4, 0:4])
            tokp = ptok.tile([128, 8], F32, tag="tok", name="tok")
            nc.vector.tensor_copy(tokp, psR)

            psAH = []
            for cc in range(2):
                p = ppt.tile([128, 256], F32, tag="ps", name=f"psAH{cc}")
                nc.tensor.matmul(p, kq0[:, 0, chs[cc]], kq0[:, :, chs[cc]],
                                 start=True, stop=False)
                nc.tensor.matmul(p, kq1[:, 0, chs[cc]], kq1[:, :, chs[cc]],
                                 start=False, stop=True)
                psAH.append(p)
            X = pcs.tile([128, 256], BF16, tag="X", name="X", bufs=2)
            Hm = pwin.tile([128, 256], BF16, tag="Hm", name="Hm")
            for cc in range(2):
                h = slice(cc * 128, (cc + 1) * 128)
                nc.vector.scalar_tensor_tensor(
                    out=X[:, h], in0=psAH[cc][:, 0:128],
                    scalar=tokp[:, cc * 4 + 1 : cc * 4 + 2], in1=mlow,
                    op0=OP.mult, op1=OP.mult,
                )
                nc.vector.tensor_mul(Hm[:, h], psAH[cc][:, 128:256], mup)

            def pairmm(lhsP, rhsP, name):
                ps = ppt.tile([128, 256], F32, tag="ps", name=name)
                for cc in range(2):
                    h = slice(cc * 128, (cc + 1) * 128)
                    nc.tensor.matmul(ps[:, h], lhsP[:, h], rhsP[:, h],
                                     start=True, stop=True)
                return ps

            psZ = ppt.tile([128, 256], BF16, tag="ps", name="psZ")
            nc.tensor.transpose(psZ[:, 0:128], X[:, 0:128], identb)
            nc.tensor.transpose(psZ[:, 128:256], X[:, 128:256], identb)
            Z = pcs.tile([128, 256], BF16, tag="Z", name="Z", bufs=2)
            nc.scalar.activation(out=Z, in_=psZ, func=AF.Copy)
            ZI = pcs.tile([128, 256], BF16, tag="ZI", name="ZI", bufs=2)
            nc.vector.tensor_add(ZI, psZ, identb2)

            psX2 = pairmm(Z, X, "psX2")
            X2 = pcs.tile([128, 256], BF16, tag="X2", name="X2", bufs=2)
            nc.vector.tensor_copy(X2, psX2)
            X2I = pcs.tile([128, 256], BF16, tag="X2I", name="X2I", bufs=2)
            nc.vector.tensor_add(X2I, psX2, identb2)

            psZ2 = pairmm(X, Z, "psZ2")
            Z2 = pcs.tile([128, 256], BF16, tag="Z2", name="Z2", bufs=2)
            nc.scalar.activation(out=Z2, in_=psZ2, func=AF.Copy)

            psX4 = pairmm(Z2, X2, "psX4")
            X4I = pcs.tile([128, 256], BF16, tag="X4I", name="X4I", bufs=2)
            nc.vector.tensor_add(X4I, psX4, identb2)

            psXB = pairmm(ZI, X2I, "psXB")
            XB = pcs.tile([128, 256], BF16, tag="XB", name="XB", bufs=2)
            nc.scalar.activation(out=XB, in_=psXB, func=AF.Copy)
            psZB = ppt.tile([128, 256], BF16, tag="ps", name="psZB")
            nc.tensor.transpose(psZB[:, 0:128], XB[:, 0:128], identb)
            nc.tensor.transpose(psZB[:, 128:256], XB[:, 128:256], identb)
            ZB = pcs.tile([128, 256], BF16, tag="ZB", name="ZB", bufs=2)
            nc.vector.tensor_copy(ZB, psZB)

            # Tt = T^T = (I+Z4)(I+Z2)(I+Z) = X4I^T @ ZB per half; used as
            # lhsT below: u = Tt^T vb = T vb, Gt = Kb^T Tt = (T Kb)^T.
            psT = pairmm(X4I, ZB, "psT")
            T = pcs.tile([128, 256], BF16, tag="T", name="T", bufs=2)
            nc.scalar.activation(out=T, in_=psT, func=AF.Copy)

            vbs, ktoks, Kbs = [], [], []
            for cc in range(2):
                ch = chs[cc]
                psVK = ppt.tile([128, 512], BF16, tag="ps", name=f"psVK{cc}")
                nc.tensor.transpose(psVK[:, 0:128], v0[:, ch], identb)
                nc.tensor.transpose(psVK[:, 128:256], v1[:, ch], identb)
                nc.tensor.transpose(psVK[:, 256:384], kq0[:, 0, ch], identb)
                nc.tensor.transpose(psVK[:, 384:512], kq1[:, 0, ch], identb)
                vb = pcm.tile([128, 256], BF16, tag="vb", name="vb", bufs=3)
                nc.vector.tensor_scalar(
                    out=vb, in0=psVK[:, 0:256],
                    scalar1=tokp[:, cc * 4 : cc * 4 + 1], scalar2=None,
                    op0=OP.mult,
                )
                ktok = pwin.tile([128, 256], BF16, tag="ktok", name="ktok",
                                 bufs=8)
                nc.scalar.activation(out=ktok, in_=psVK[:, 256:512], func=AF.Copy)
                vbs.append(vb)
                ktoks.append(ktok)
                if 2 * pi + cc > 0:
                    Kb = pcm.tile([128, 256], BF16, tag="Kb", name="Kb", bufs=3)
                    nc.vector.tensor_scalar(
                        out=Kb, in0=psVK[:, 256:512],
                        scalar1=tokp[:, cc * 4 + 1 : cc * 4 + 2], scalar2=None,
                        op0=OP.mult,
                    )
                    Kbs.append(Kb)
                else:
                    Kbs.append(None)

            psU = ppt.tile([128, 512], F32, tag="ps", name="psU")
            nc.tensor.matmul(psU[:, 0:256], T[:, 0:128], vbs[0],
                             start=True, stop=True)
            nc.tensor.matmul(psU[:, 256:512], T[:, 128:256], vbs[1],
                             start=True, stop=True)
            u = pwin.tile([128, 512], BF16, tag="u", name="u")
            nc.scalar.activation(out=u, in_=psU, func=AF.Copy)

            psGt = ppt.tile([128, 512], F32, tag="ps", name="psGt")
            for cc in range(2):
                if Kbs[cc] is None:
                    continue
                for dh in range(2):
                    nc.tensor.matmul(
                        psGt[:, cc * 256 + dh * 128 : cc * 256 + (dh + 1) * 128],
                        Kbs[cc][:, dh * 128 : (dh + 1) * 128],
                        T[:, cc * 128 : (cc + 1) * 128],
                        start=True, stop=True,
                    )
            Gt = pwin.tile([128, 512], BF16, tag="Gt", name="Gt")
            if pi == 0:
                nc.scalar.activation(out=Gt[:, 256:512], in_=psGt[:, 256:512],
                                     func=AF.Copy)
            else:
                nc.scalar.activation(out=Gt, in_=psGt, func=AF.Copy)

            for cc in range(2):
                i = 2 * pi + cc
                state[i] = dict(
                    aq_t=tokp[:, cc * 4 + 2 : cc * 4 + 3],
                    aq2_t=tokp[:, cc * 4 + 3 : cc * 4 + 4],
                    Hm=Hm[:, cc * 128 : (cc + 1) * 128],
                    ktok=ktoks[cc],
                    u=u[:, cc * 256 : (cc + 1) * 256],
                    Gt0=Gt[:, cc * 256 : cc * 256 + 128],
                    Gt1=Gt[:, cc * 256 + 128 : (cc + 1) * 256],
                )

        # ---------------- serial state chain ----------------
        def emit_chain(i):
            ch = slice(i * C, (i + 1) * C)
            st = state[i]
            S_sb = None
            if i > 0:
                S_sb = pS.tile([128, 512], BF16, tag="S", name="S_sb")
                nc.scalar.activation(out=S_sb[:, 0:256], in_=psS0, func=AF.Copy)
                nc.vector.tensor_copy(S_sb[:, 256:512], psS1)

            if i > 0:
                psW = ppw.tile([128, 256], F32, tag="cw", name="psW")
                nc.tensor.matmul(psW, st["Gt0"], S_sb[:, 0:256],
                                 start=True, stop=False)
                nc.tensor.matmul(psW, st["Gt1"], S_sb[:, 256:512],
                                 start=False, stop=True)
                w = pcm.tile([128, 256], BF16, tag="w", name="w", bufs=3)
                nc.vector.tensor_add(w, psW, st["u"])
            else:
                w = st["u"]

            # state update first so the chain keeps moving
            nc.tensor.matmul(
                psS0, st["ktok"][:, 0:128], w,
                start=(i == 0), stop=(i == nchunk - 1), skip_group_check=True,
            )
            nc.tensor.matmul(
                psS1, st["ktok"][:, 128:256], w,
                start=(i == 0), stop=(i == nchunk - 1), skip_group_check=True,
            )

            pso = ppo.tile([128, 256], F32, tag="po", name="pso")
            if i > 0:
                nc.tensor.matmul(pso, kq0[:, 1, ch], S_sb[:, 0:256],
                                 start=True, stop=False)
                nc.tensor.matmul(pso, kq1[:, 1, ch], S_sb[:, 256:512],
                                 start=False, stop=False)
                nc.tensor.matmul(pso, st["Hm"], w, start=False, stop=True)
            else:
                nc.tensor.matmul(pso, st["Hm"], w, start=True, stop=True)
            st["pso"] = pso

        # ---------------- trailing: rms-norm + o_proj ----------------
        def emit_trailing(i):
            ch = slice(i * C, (i + 1) * C)
            st = state.pop(i)
            pso = st["pso"]

            o_sb = pcm.tile([128, 256], BF16, tag="o_sb", name="o_sb", bufs=2)
            nc.vector.tensor_copy(o_sb, pso)
            psOT = ppt.tile([128, 256], BF16, tag="ps", name="psOT")
            nc.tensor.transpose(psOT[:, 0:128], o_sb[:, 0:128], identb)
            nc.tensor.transpose(psOT[:, 128:256], o_sb[:, 128:256], identb)
            ot = pcm.tile([128, 256], BF16, tag="ot", name="ot", bufs=2)
            nc.vector.tensor_copy(ot, psOT)

            # rms sums via Act square-with-accumulate (table-set resident)
            sums = ptok.tile([128, 1], F32, tag="sums", name="sums")
            scratch = pcm.tile([128, 256], F32, tag="scr2", name="scratch",
                               bufs=2)
            nc.scalar.activation(
                out=scratch, in_=o_sb, func=AF.Square, accum_out=sums
            )
            rs = pcs.tile([128, 3], F32, tag="rs", name="rs", bufs=2)
            nc.scalar.activation(
                out=rs[:, 0:1], in_=sums, func=AF.Sqrt,
                scale=st["aq2_t"], bias=eps5,
            )
            nc.vector.reciprocal(out=rs[:, 2:3], in_=rs[:, 0:1])
            nc.vector.tensor_mul(rs[:, 1:2], rs[:, 2:3], st["aq_t"])

            outbuf = pout.tile([128, HID], F32, tag="outbuf", name="outbuf")
            for hc in range(2):
                psop = ppt.tile([128, 512], F32, tag="ps", name="psop")
                nc.tensor.matmul(
                    psop, ot[:, 0:128], wo_sb[:, 0, hc * 512 : (hc + 1) * 512],
                    start=True, stop=False,
                )
                nc.tensor.matmul(
                    psop, ot[:, 128:256], wo_sb[:, 1, hc * 512 : (hc + 1) * 512],
                    start=False, stop=True,
                )
                dst = outbuf[:, hc * 512 : (hc + 1) * 512]
                if hc == 0:
                    nc.vector.tensor_scalar(
                        out=dst, in0=psop, scalar1=rs[:, 1:2], scalar2=None,
                        op0=OP.mult,
                    )
                else:
                    nc.scalar.activation(
                        out=dst, in_=psop, func=AF.Copy, scale=rs[:, 1:2]
                    )
            nc.gpsimd.dma_start(out=out_d.ap()[ch, :], in_=outbuf)

        # ---------------- slab-interleaved driver ----------------
        for sc in range(nsc):
            emit_B_slab(sc)
            emit_rows(sc)
            emit_pass1_pair(2 * sc)
            emit_pass1_pair(2 * sc + 1)
            if sc >= 1:
                for i in range(cps * (sc - 1), cps * sc):
                    emit_chain(i)
                    if i > 0:
                        emit_trailing(i - 1)
        for i in range(cps * (nsc - 1), nchunk):
            emit_chain(i)
            emit_trailing(i - 1)
        emit_trailing(nchunk - 1)

    nc.compile()
    return nc


def make_host_inputs(inputs, nchunk=S_FULL // C):
    S = nchunk * C
    bf = ml_dtypes.bfloat16
    hs = np.asarray(inputs["hidden_states"], np.float32)[:, :S, :]
    Wq, Wk, Wv = (np.asarray(inputs[k], np.float32) for k in ("Wq", "Wk", "Wv"))
    Wb = np.asarray(inputs["Wb"], np.float32)
    Wo = np.asarray(inputs["Wo"], np.float32)
    nw = np.asarray(inputs["norm_w"], np.float32)
    convs = {
        k: np.asarray(inputs[k], np.float32) for k in ("conv_q", "conv_k", "conv_v")
    }

    identb = np.eye(128, dtype=np.float32)
    onescol = np.ones((128, 1), np.float32)
    mlow = np.tril(np.ones((128, 128), np.float32), -1)
    mup = np.triu(np.ones((128, 128), np.float32), 0)

    def diag_pack(cw):
        out = np.zeros((128, 8 * 128), np.float32)
        for j in range(4):
            for dt_ in range(2):
                blk = np.diag(cw[dt_ * 128 : (dt_ + 1) * 128, j])
                out[:, (j * 2 + dt_) * 128 : (j * 2 + dt_ + 1) * 128] = blk
        return out

    def c(a, dt=bf):
        return np.ascontiguousarray(a).astype(dt)

    in_maps = []
    for core in range(8):
        b, h = core // 4, core % 4
        hsel = slice(h * D, (h + 1) * D)
        in_maps.append(
            {
                "xt": c(hs[b].T),
                "wq": c(Wq[:, hsel]),
                "wk": c(Wk[:, hsel]),
                "wv": c(Wv[:, hsel]),
                "wb": c(Wb[:, h : h + 1]),
                "wo": c(nw[:, None] * Wo[hsel, :]),
                "cdq": c(diag_pack(convs["conv_q"][hsel])),
                "cdk": c(diag_pack(convs["conv_k"][hsel])),
                "cdv": c(diag_pack(convs["conv_v"][hsel])),
                "identb": c(identb),
                "identf": c(identb, np.float32),
                "identb2": c(np.concatenate([identb, identb], axis=1)),
                "onescol": c(onescol),
                "mlow": c(mlow, np.float32),
                "mup": c(mup, np.float32),
            }
        )
    return in_maps


_NC_CACHE = {}


def _get_nc(nchunk):
    if nchunk not in _NC_CACHE:
        _NC_CACHE[nchunk] = build_nc(nchunk)
    return _NC_CACHE[nchunk]


def kernel(**inputs) -> np.ndarray:
    nchunk = S_FULL // C
    nc = _get_nc(nchunk)
    in_maps = make_host_inputs(inputs, nchunk)
    res = run_bass_kernel_spmd(nc, in_maps, core_ids=list(range(8)))
    S = nchunk * C
    out = np.zeros((B, S, HID), np.float32)
    for core in range(8):
        out[core // 4] += np.asarray(res.results[core]["out"], np.float32)
    return out


# revision 42
# speedup vs baseline: 1.1732x; 1.1732x over previous
"""DeltaNet Trainium2 kernel — 8-core SPMD, one (batch, head) pair per core.

Slab-interleaved pipeline: for each 512-token slab, emit projections+conv+silu
(PE-dense), then the token-scalar rows (beta/l2 via Act/DVE), then the two
chunk-pair UT-transform ladders, then the previous slab's serial state chains
and o_proj trailing work.  bf16 datapath with fp32 PSUM accumulation; the
chunked delta rule precomputes Tt = ((I+X)(I+X^2)(I+X^4))^T per chunk off the
serial chain, with both chunks of a pair sharing PSUM/SBUF pair tiles (all
ladder products are single-shot matmuls).  Host folds norm_w into Wo and sums
the 4 per-head partial o_proj outputs per batch.
"""

import os
import sys
from contextlib import ExitStack

import ml_dtypes
import numpy as np

for _p in ("/opt/trn_rl_repo", "/root/.axon_site/_ro/trn_rl_repo"):
    if os.path.isdir(_p) and _p not in sys.path:
        sys.path.insert(0, _p)

import concourse.bass as bass  # noqa: E402
import concourse.tile as tile  # noqa: E402
from concourse import bacc, mybir  # noqa: E402
from concourse.bass_utils import run_bass_kernel_spmd  # noqa: E402

F32 = mybir.dt.float32
F32R = mybir.dt.float32r
BF16 = mybir.dt.bfloat16
AF = mybir.ActivationFunctionType
OP = mybir.AluOpType

HID = 1024
D = 256
C = 128
KT = HID // 128
NH = 4
B = 2
S_FULL = 2048


def build_nc(nchunk=S_FULL // C, dbg=False):
    S = nchunk * C
    scs = 512 if S >= 512 else S
    nsc = S // scs
    cps = scs // C  # chunks per slab (4)
    nc = bacc.Bacc("TRN2", target_bir_lowering=False, debug=False)

    xt_d = nc.dram_tensor("xt", [HID, S], BF16, kind="ExternalInput")
    wq_d = nc.dram_tensor("wq", [HID, D], BF16, kind="ExternalInput")
    wk_d = nc.dram_tensor("wk", [HID, D], BF16, kind="ExternalInput")
    wv_d = nc.dram_tensor("wv", [HID, D], BF16, kind="ExternalInput")
    wb_d = nc.dram_tensor("wb", [HID, 1], BF16, kind="ExternalInput")
    wo_d = nc.dram_tensor("wo", [D, HID], BF16, kind="ExternalInput")
    cdq_d = nc.dram_tensor("cdq", [128, 8 * 128], BF16, kind="ExternalInput")
    cdk_d = nc.dram_tensor("cdk", [128, 8 * 128], BF16, kind="ExternalInput")
    cdv_d = nc.dram_tensor("cdv", [128, 8 * 128], BF16, kind="ExternalInput")
    identb_d = nc.dram_tensor("identb", [128, 128], BF16, kind="ExternalInput")
    identf_d = nc.dram_tensor("identf", [128, 128], F32R, kind="ExternalInput")
    identb2_d = nc.dram_tensor("identb2", [128, 256], BF16, kind="ExternalInput")
    onescol_d = nc.dram_tensor("onescol", [128, 1], BF16, kind="ExternalInput")
    mlow_d = nc.dram_tensor("mlow", [128, 128], F32, kind="ExternalInput")
    mup_d = nc.dram_tensor("mup", [128, 128], F32, kind="ExternalInput")
    out_d = nc.dram_tensor("out", [S, HID], F32, kind="ExternalOutput")

    with tile.TileContext(nc) as tc, ExitStack() as ctx:
        # ---------------- pools ----------------
        pconst = ctx.enter_context(tc.tile_pool(name="pconst", bufs=1))
        pplane = ctx.enter_context(tc.tile_pool(name="pplane", bufs=1))
        pw = ctx.enter_context(tc.tile_pool(name="pw", bufs=1))
        pxt = ctx.enter_context(tc.tile_pool(name="pxt", bufs=1))
        praw = ctx.enter_context(tc.tile_pool(name="praw", bufs=1))
        prow = ctx.enter_context(tc.tile_pool(name="prow", bufs=2))
        pcsq = ctx.enter_context(tc.tile_pool(name="pcsq", bufs=3))
        pS = ctx.enter_context(tc.tile_pool(name="pS", bufs=2))
        pcs = ctx.enter_context(tc.tile_pool(name="pcs", bufs=2))
        pcm = ctx.enter_context(tc.tile_pool(name="pcm", bufs=2))
        pwin = ctx.enter_context(tc.tile_pool(name="pwin", bufs=4))
        ptok = ctx.enter_context(tc.tile_pool(name="ptok", bufs=nchunk))
        pout = ctx.enter_context(tc.tile_pool(name="pout", bufs=2))
        # PSUM: 8 banks total (2 state + 1 chain + 1 pso + 2 B + 2 ladder)
        ppS = ctx.enter_context(tc.tile_pool(name="ppS", bufs=1, space="PSUM"))
        ppw = ctx.enter_context(tc.tile_pool(name="ppw", bufs=1, space="PSUM"))
        ppo = ctx.enter_context(tc.tile_pool(name="ppo", bufs=1, space="PSUM"))
        ppt = ctx.enter_context(tc.tile_pool(name="ppt", bufs=2, space="PSUM"))
        ppb = ctx.enter_context(tc.tile_pool(name="ppb", bufs=2, space="PSUM"))

        identb = pconst.tile([128, 128], BF16)
        identb2 = pconst.tile([128, 256], BF16)
        identf = pconst.tile([128, 128], F32R)
        onescol = pconst.tile([128, 1], BF16)
        rows4 = pconst.tile([4, S], F32R)  # 0=bk, 1=nbk2, 2=aq, 3=aq2
        mlow = pconst.tile([128, 128], F32)
        mup = pconst.tile([128, 128], F32)
        eps6 = pconst.tile([128, 1], F32)
        eps5 = pconst.tile([128, 1], F32)
        nc.vector.memset(eps6, 1e-6)
        nc.vector.memset(eps5, 1e-5)

        wo_sb = pw.tile([128, 2, HID], BF16)
        wb_sb = pw.tile([128, KT, 1], BF16)

        # planes: kq{dt} holds k in [:,0,:] and q in [:,1,:]; v separate
        kq0 = pplane.tile([128, 2, S], BF16, name="kq0")
        kq1 = pplane.tile([128, 2, S], BF16, name="kq1")
        v0 = pplane.tile([128, S], BF16, name="v0")
        v1 = pplane.tile([128, S], BF16, name="v1")

        xt_sb = pxt.tile([128, KT, S], BF16)

        # ---------------- input DMAs (slab-major xt for early start) -------
        nc.sync.dma_start(
            out=wb_sb, in_=wb_d.ap().rearrange("(k p) o -> p k o", p=128)
        )
        wd_srcs = {"q": wq_d, "k": wk_d, "v": wv_d}
        w_sbs = {}
        for t in ("k", "q", "v"):
            w_sbs[t] = pw.tile([128, KT, D], BF16, name=f"w_{t}")
            nc.sync.dma_start(
                out=w_sbs[t],
                in_=wd_srcs[t].ap().rearrange("(k p) d -> p k d", p=128),
            )
        xt_src = xt_d.ap().rearrange("(k p) s -> p k s", p=128)
        for sc in range(nsc):
            sl = slice(sc * scs, (sc + 1) * scs)
            for kk in range(KT):
                nc.sync.dma_start(out=xt_sb[:, kk, sl], in_=xt_src[:, kk, sl])
        diags = {}
        for t, cd_d in (("k", cdk_d), ("q", cdq_d), ("v", cdv_d)):
            diags[t] = pw.tile([128, 8 * 128], BF16, name=f"diag_{t}")
            nc.sync.dma_start(out=diags[t], in_=cd_d.ap())
        nc.gpsimd.dma_start(out=identb, in_=identb_d.ap())
        nc.gpsimd.dma_start(out=identb2, in_=identb2_d.ap())
        nc.gpsimd.dma_start(out=identf, in_=identf_d.ap())
        nc.gpsimd.dma_start(out=onescol, in_=onescol_d.ap())
        nc.gpsimd.dma_start(out=mlow, in_=mlow_d.ap())
        nc.gpsimd.dma_start(out=mup, in_=mup_d.ap())
        nc.sync.dma_start(
            out=wo_sb, in_=wo_d.ap().rearrange("(t p) h -> p t h", p=128)
        )

        raws = {}
        for t in ("k", "q", "v"):
            for dt_ in range(2):
                raws[(t, dt_)] = praw.tile(
                    [128, S + 8], BF16, name=f"raw_{t}{dt_}"
                )
                nc.gpsimd.memset(raws[(t, dt_)][:, 0:8], 0.0)

        copy_flip = [0]

        # ---------------- phase B slab: proj + conv + silu ----------------
        def emit_B_slab(sc):
            base = sc * scs
            for t in ("k", "q", "v"):
                diag = diags[t]
                for dt_ in range(2):
                    raw = raws[(t, dt_)]
                    ps = ppb.tile([128, scs], F32, tag="ps", name="psraw")
                    for kk in range(KT):
                        nc.tensor.matmul(
                            ps,
                            w_sbs[t][:, kk, dt_ * 128 : (dt_ + 1) * 128],
                            xt_sb[:, kk, base : base + scs],
                            start=(kk == 0),
                            stop=(kk == KT - 1),
                        )
                    dst = raw[:, 8 + base : 8 + base + scs]
                    if copy_flip[0] % 2 == 0:
                        nc.vector.tensor_copy(dst, ps)
                    else:
                        nc.scalar.activation(out=dst, in_=ps, func=AF.Copy)
                    copy_flip[0] += 1
                    if t == "v":
                        pdst = (v0, v1)[dt_][:, base : base + scs]
                    else:
                        pdst = (kq0, kq1)[dt_][
                            :, 0 if t == "k" else 1, base : base + scs
                        ]
                    psc = ppb.tile([128, scs], F32, tag="ps", name="psconv")
                    for j in (3, 2, 1, 0):
                        sh = 3 - j
                        dslc = diag[
                            :, (j * 2 + dt_) * 128 : (j * 2 + dt_ + 1) * 128
                        ]
                        nc.tensor.matmul(
                            psc,
                            dslc,
                            raw[:, 8 + base - sh : 8 + base + scs - sh],
                            start=(j == 3),
                            stop=(j == 0),
                        )
                    nc.scalar.activation(out=pdst, in_=psc, func=AF.Silu)

        # ---------------- token-scalar rows for one slab ----------------
        def emit_rows(sc):
            sl = slice(sc * scs, (sc + 1) * scs)
            psb = ppb.tile([1, scs], F32, tag="ps", name="psb")
            for kk in range(KT):
                nc.tensor.matmul(
                    psb, wb_sb[:, kk, :], xt_sb[:, kk, sl],
                    start=(kk == 0), stop=(kk == KT - 1),
                )
            beta_row = prow.tile([1, scs], F32, tag="beta", name="beta_row")
            nc.scalar.activation(out=beta_row, in_=psb, func=AF.Sigmoid)
            for t in ("k", "q"):
                ti = 0 if t == "k" else 1
                psl = ppb.tile([1, scs], F32, tag="ps", name="psl")
                for dt_, kqp in ((0, kq0), (1, kq1)):
                    sq = pcsq.tile([128, scs], BF16, tag="sq", name="sq")
                    nc.vector.tensor_mul(sq, kqp[:, ti, sl], kqp[:, ti, sl])
                    nc.tensor.matmul(
                        psl, onescol, sq, start=(dt_ == 0), stop=(dt_ == 1)
                    )
                srow = prow.tile([1, scs], F32, tag="srow", name="srow")
                nc.scalar.activation(
                    out=srow, in_=psl, func=AF.Sqrt, bias=eps6[0:1, :]
                )
                with nc.allow_low_precision(reason="f32r tok rows"):
                    if t == "k":
                        ak = prow.tile([1, scs], F32, tag="ak", name="ak")
                        bk = prow.tile([1, scs], F32R, tag="bk", name="bk")
                        nb = prow.tile([1, scs], F32R, tag="nb", name="nb")
                        nc.vector.reciprocal(out=ak, in_=srow)
                        nc.vector.tensor_mul(bk, beta_row, ak)
                        nc.vector.scalar_tensor_tensor(
                            out=nb, in0=bk, scalar=-1.0, in1=ak,
                            op0=OP.mult, op1=OP.mult,
                        )
                        nc.sync.dma_start(out=rows4[0:1, sl], in_=bk)
                        nc.sync.dma_start(out=rows4[1:2, sl], in_=nb)
                    else:
                        aqr = prow.tile([1, scs], F32R, tag="aqr", name="aqr")
                        aq2 = prow.tile([1, scs], F32R, tag="aq2", name="aq2")
                        nc.vector.reciprocal(out=aqr, in_=srow)
                        nc.vector.scalar_tensor_tensor(
                            out=aq2, in0=aqr, scalar=1.0 / D, in1=aqr,
                            op0=OP.mult, op1=OP.mult,
                        )
                        nc.sync.dma_start(out=rows4[2:3, sl], in_=aqr)
                        nc.sync.dma_start(out=rows4[3:4, sl], in_=aq2)

        psS0 = ppS.tile([128, 256], F32, tag="psS0", name="psS0")
        psS1 = ppS.tile([128, 256], F32, tag="psS1", name="psS1")

        state = {}

        # ---------------- pass 1: chunk-pair UT-transform ladder ----------
        def emit_pass1_pair(pi):
            ca, cb = 2 * pi, 2 * pi + 1
            chs = [slice(ca * C, (ca + 1) * C), slice(cb * C, (cb + 1) * C)]

            psR = ppt.tile([128, 8], F32R, tag="ps", name="psR")
            nc.tensor.transpose(psR[:, 0:4], rows4[0:4, chs[0]], identf[0:4, 0:4])
            nc.tensor.transpose(psR[:, 4:8], rows4[0:4, chs[1]], identf[0:4, 0:4])
            tokp = ptok.tile([128, 8], F32, tag="tok", name="tok")
            nc.vector.tensor_copy(tokp, psR)

            psAH = []
            for cc in range(2):
                p = ppt.tile([128, 256], F32, tag="ps", name=f"psAH{cc}")
                nc.tensor.matmul(p, kq0[:, 0, chs[cc]], kq0[:, :, chs[cc]],
                                 start=True, stop=False)
                nc.tensor.matmul(p, kq1[:, 0, chs[cc]], kq1[:, :, chs[cc]],
                                 start=False, stop=True)
                psAH.append(p)
            X = pcs.tile([128, 256], BF16, tag="X", name="X", bufs=2)
            Hm = pwin.tile([128, 256], BF16, tag="Hm", name="Hm")
            for cc in range(2):
                h = slice(cc * 128, (cc + 1) * 128)
                nc.vector.scalar_tensor_tensor(
                    out=X[:, h], in0=psAH[cc][:, 0:128],
                    scalar=tokp[:, cc * 4 + 1 : cc * 4 + 2], in1=mlow,
                    op0=OP.mult, op1=OP.mult,
                )
                nc.vector.tensor_mul(Hm[:, h], psAH[cc][:, 128:256], mup)

            def pairmm(lhsP, rhsP, name):
                ps = ppt.tile([128, 256], F32, tag="ps", name=name)
                for cc in range(2):
                    h = slice(cc * 128, (cc + 1) * 128)
                    nc.tensor.matmul(ps[:, h], lhsP[:, h], rhsP[:, h],
                                     start=True, stop=True)
                return ps

            psZ = ppt.tile([128, 256], BF16, tag="ps", name="psZ")
            nc.tensor.transpose(psZ[:, 0:128], X[:, 0:128], identb)
            nc.tensor.transpose(psZ[:, 128:256], X[:, 128:256], identb)
            Z = pcs.tile([128, 256], BF16, tag="Z", name="Z", bufs=2)
            nc.scalar.activation(out=Z, in_=psZ, func=AF.Copy)
            ZI = pcs.tile([128, 256], BF16, tag="ZI", name="ZI", bufs=2)
            nc.vector.tensor_add(ZI, psZ, identb2)

            psX2 = pairmm(Z, X, "psX2")
            X2 = pcs.tile([128, 256], BF16, tag="X2", name="X2", bufs=2)
            nc.vector.tensor_copy(X2, psX2)
            X2I = pcs.tile([128, 256], BF16, tag="X2I", name="X2I", bufs=2)
            nc.vector.tensor_add(X2I, psX2, identb2)

            psZ2 = pairmm(X, Z, "psZ2")
            Z2 = pcs.tile([128, 256], BF16, tag="Z2", name="Z2", bufs=2)
            nc.scalar.activation(out=Z2, in_=psZ2, func=AF.Copy)

            psX4 = pairmm(Z2, X2, "psX4")
            X4I = pcs.tile([128, 256], BF16, tag="X4I", name="X4I", bufs=2)
            nc.vector.tensor_add(X4I, psX4, identb2)

            psXB = pairmm(ZI, X2I, "psXB")
            XB = pcs.tile([128, 256], BF16, tag="XB", name="XB", bufs=2)
            nc.scalar.activation(out=XB, in_=psXB, func=AF.Copy)
            psZB = ppt.tile([128, 256], BF16, tag="ps", name="psZB")
            nc.tensor.transpose(psZB[:, 0:128], XB[:, 0:128], identb)
            nc.tensor.transpose(psZB[:, 128:256], XB[:, 128:256], identb)
            ZB = pcs.tile([128, 256], BF16, tag="ZB", name="ZB", bufs=2)
            nc.vector.tensor_copy(ZB, psZB)

            # Tt = T^T = (I+Z4)(I+Z2)(I+Z) = X4I^T @ ZB per half; used as
            # lhsT below: u = Tt^T vb = T vb, Gt = Kb^T Tt = (T Kb)^T.
            psT = pairmm(X4I, ZB, "psT")
            T = pcs.tile([128, 256], BF16, tag="T", name="T", bufs=2)
            nc.scalar.activation(out=T, in_=psT, func=AF.Copy)

            vbs, ktoks, Kbs = [], [], []
            for cc in range(2):
                ch = chs[cc]
                psVK = ppt.tile([128, 512], BF16, tag="ps", name=f"psVK{cc}")
                nc.tensor.transpose(psVK[:, 0:128], v0[:, ch], identb)
                nc.tensor.transpose(psVK[:, 128:256], v1[:, ch], identb)
                nc.tensor.transpose(psVK[:, 256:384], kq0[:, 0, ch], identb)
                nc.tensor.transpose(psVK[:, 384:512], kq1[:, 0, ch], identb)
                vb = pcm.tile([128, 256], BF16, tag="vb", name="vb", bufs=3)
                nc.vector.tensor_scalar(
                    out=vb, in0=psVK[:, 0:256],
                    scalar1=tokp[:, cc * 4 : cc * 4 + 1], scalar2=None,
                    op0=OP.mult,
                )
                ktok = pwin.tile([128, 256], BF16, tag="ktok", name="ktok",
                                 bufs=8)
                nc.scalar.activation(out=ktok, in_=psVK[:, 256:512], func=AF.Copy)
                vbs.append(vb)
                ktoks.append(ktok)
                if 2 * pi + cc > 0:
                    Kb = pcm.tile([128, 256], BF16, tag="Kb", name="Kb", bufs=3)
                    nc.vector.tensor_scalar(
                        out=Kb, in0=psVK[:, 256:512],
                        scalar1=tokp[:, cc * 4 + 1 : cc * 4 + 2], scalar2=None,
                        op0=OP.mult,
                    )
                    Kbs.append(Kb)
                else:
                    Kbs.append(None)

            psU = ppt.tile([128, 512], F32, tag="ps", name="psU")
            nc.tensor.matmul(psU[:, 0:256], T[:, 0:128], vbs[0],
                             start=True, stop=True)
            nc.tensor.matmul(psU[:, 256:512], T[:, 128:256], vbs[1],
                             start=True, stop=True)
            u = pwin.tile([128, 512], BF16, tag="u", name="u")
            nc.scalar.activation(out=u, in_=psU, func=AF.Copy)

            psGt = ppt.tile([128, 512], F32, tag="ps", name="psGt")
            for cc in range(2):
                if Kbs[cc] is None:
                    continue
                for dh in range(2):
                    nc.tensor.matmul(
                        psGt[:, cc * 256 + dh * 128 : cc * 256 + (dh + 1) * 128],
                        Kbs[cc][:, dh * 128 : (dh + 1) * 128],
                        T[:, cc * 128 : (cc + 1) * 128],
                        start=True, stop=True,
                    )
            Gt = pwin.tile([128, 512], BF16, tag="Gt", name="Gt")
            if pi == 0:
                nc.scalar.activation(out=Gt[:, 256:512], in_=psGt[:, 256:512],
                                     func=AF.Copy)
            else:
                nc.scalar.activation(out=Gt, in_=psGt, func=AF.Copy)

            for cc in range(2):
                i = 2 * pi + cc
                state[i] = dict(
                    aq_t=tokp[:, cc * 4 + 2 : cc * 4 + 3],
                    aq2_t=tokp[:, cc * 4 + 3 : cc * 4 + 4],
                    Hm=Hm[:, cc * 128 : (cc + 1) * 128],
                    ktok=ktoks[cc],
                    u=u[:, cc * 256 : (cc + 1) * 256],
                    Gt0=Gt[:, cc * 256 : cc * 256 + 128],
                    Gt1=Gt[:, cc * 256 + 128 : (cc + 1) * 256],
                )

        # ---------------- serial state chain ----------------
        def emit_chain(i):
            ch = slice(i * C, (i + 1) * C)
            st = state[i]
            S_sb = None
            if i > 0:
                S_sb = pS.tile([128, 512], BF16, tag="S", name="S_sb")
                nc.scalar.activation(out=S_sb[:, 0:256], in_=psS0, func=AF.Copy)
                nc.vector.tensor_copy(S_sb[:, 256:512], psS1)

            if i > 0:
                psW = ppw.tile([128, 256], F32, tag="cw", name="psW")
                nc.tensor.matmul(psW, st["Gt0"], S_sb[:, 0:256],
                                 start=True, stop=False)
                nc.tensor.matmul(psW, st["Gt1"], S_sb[:, 256:512],
                                 start=False, stop=True)
                w = pcm.tile([128, 256], BF16, tag="w", name="w", bufs=3)
                nc.vector.tensor_add(w, psW, st["u"])
            else:
                w = st["u"]

            # state update first so the chain keeps moving
            nc.tensor.matmul(
                psS0, st["ktok"][:, 0:128], w,
                start=(i == 0), stop=(i == nchunk - 1), skip_group_check=True,
            )
            nc.tensor.matmul(
                psS1, st["ktok"][:, 128:256], w,
                start=(i == 0), stop=(i == nchunk - 1), skip_group_check=True,
            )

            pso = ppo.tile([128, 256], F32, tag="po", name="pso")
            if i > 0:
                nc.tensor.matmul(pso, kq0[:, 1, ch], S_sb[:, 0:256],
                                 start=True, stop=False)
                nc.tensor.matmul(pso, kq1[:, 1, ch], S_sb[:, 256:512],
                                 start=False, stop=False)
                nc.tensor.matmul(pso, st["Hm"], w, start=False, stop=True)
            else:
                nc.tensor.matmul(pso, st["Hm"], w, start=True, stop=True)
            st["pso"] = pso

        # ---------------- trailing: rms-norm + o_proj ----------------
        def emit_trailing(i):
            ch = slice(i * C, (i + 1) * C)
            st = state.pop(i)
            pso = st["pso"]

            o_sb = pcm.tile([128, 256], BF16, tag="o_sb", name="o_sb", bufs=2)
            nc.vector.tensor_copy(o_sb, pso)
            psOT = ppt.tile([128, 256], BF16, tag="ps", name="psOT")
            nc.tensor.transpose(psOT[:, 0:128], o_sb[:, 0:128], identb)
            nc.tensor.transpose(psOT[:, 128:256], o_sb[:, 128:256], identb)
            ot = pcm.tile([128, 256], BF16, tag="ot", name="ot", bufs=2)
            nc.vector.tensor_copy(ot, psOT)

            # rms sums via Act square-with-accumulate (table-set resident)
            sums = ptok.tile([128, 1], F32, tag="sums", name="sums")
            scratch = pcm.tile([128, 256], F32, tag="scr2", name="scratch",
                               bufs=2)
            nc.scalar.activation(
                out=scratch, in_=o_sb, func=AF.Square, accum_out=sums
            )
            rs = pcs.tile([128, 3], F32, tag="rs", name="rs", bufs=2)
            nc.scalar.activation(
                out=rs[:, 0:1], in_=sums, func=AF.Sqrt,
                scale=st["aq2_t"], bias=eps5,
            )
            nc.vector.reciprocal(out=rs[:, 2:3], in_=rs[:, 0:1])
            nc.vector.tensor_mul(rs[:, 1:2], rs[:, 2:3], st["aq_t"])

            outbuf = pout.tile([128, HID], F32, tag="outbuf", name="outbuf")
            for hc in range(2):
                psop = ppt.tile([128, 512], F32, tag="ps", name="psop")
                nc.tensor.matmul(
                    psop, ot[:, 0:128], wo_sb[:, 0, hc * 512 : (hc + 1) * 512],
                    start=True, stop=False,
                )
                nc.tensor.matmul(
                    psop, ot[:, 128:256], wo_sb[:, 1, hc * 512 : (hc + 1) * 512],
                    start=False, stop=True,
                )
                dst = outbuf[:, hc * 512 : (hc + 1) * 512]
                if hc == 0:
                    nc.vector.tensor_scalar(
                        out=dst, in0=psop, scalar1=rs[:, 1:2], scalar2=None,
                        op0=OP.mult,
                    )
                else:
                    nc.scalar.activation(
                        out=dst, in_=psop, func=AF.Copy, scale=rs[:, 1:2]
                    )
            nc.gpsimd.dma_start(out=out_d.ap()[ch, :], in_=outbuf)

        # ---------------- slab-interleaved driver ----------------
        for sc in range(nsc):
            emit_B_slab(sc)
            emit_rows(sc)
            emit_pass1_pair(2 * sc)
            emit_pass1_pair(2 * sc + 1)
            if sc >= 1:
                for i in range(cps * (sc - 1), cps * sc):
                    emit_chain(i)
                    if i > 0:
                        emit_trailing(i - 1)
        for i in range(cps * (nsc - 1), nchunk):
            emit_chain(i)
            emit_trailing(i - 1)
        emit_trailing(nchunk - 1)

    nc.compile()
    return nc


def make_host_inputs(inputs, nchunk=S_FULL // C):
    S = nchunk * C
    bf = ml_dtypes.bfloat16
    hs = np.asarray(inputs["hidden_states"], np.float32)[:, :S, :]
    Wq, Wk, Wv = (np.asarray(inputs[k], np.float32) for k in ("Wq", "Wk", "Wv"))
    Wb = np.asarray(inputs["Wb"], np.float32)
    Wo = np.asarray(inputs["Wo"], np.float32)
    nw = np.asarray(inputs["norm_w"], np.float32)
    convs = {
        k: np.asarray(inputs[k], np.float32) for k in ("conv_q", "conv_k", "conv_v")
    }

    identb = np.eye(128, dtype=np.float32)
    onescol = np.ones((128, 1), np.float32)
    mlow = np.tril(np.ones((128, 128), np.float32), -1)
    mup = np.triu(np.ones((128, 128), np.float32), 0)

    def diag_pack(cw):
        out = np.zeros((128, 8 * 128), np.float32)
        for j in range(4):
            for dt_ in range(2):
                blk = np.diag(cw[dt_ * 128 : (dt_ + 1) * 128, j])
                out[:, (j * 2 + dt_) * 128 : (j * 2 + dt_ + 1) * 128] = blk
        return out

    def c(a, dt=bf):
        return np.ascontiguousarray(a).astype(dt)

    in_maps = []
    for core in range(8):
        b, h = core // 4, core % 4
        hsel = slice(h * D, (h + 1) * D)
        in_maps.append(
            {
                "xt": c(hs[b].T),
                "wq": c(Wq[:, hsel]),
                "wk": c(Wk[:, hsel]),
                "wv": c(Wv[:, hsel]),
                "wb": c(Wb[:, h : h + 1]),
                "wo": c(nw[:, None] * Wo[hsel, :]),
                "cdq": c(diag_pack(convs["conv_q"][hsel])),
                "cdk": c(diag_pack(convs["conv_k"][hsel])),
                "cdv": c(diag_pack(convs["conv_v"][hsel])),
                "identb": c(identb),
                "identf": c(identb, np.float32),
                "identb2": c(np.concatenate([identb, identb], axis=1)),
                "onescol": c(onescol),
                "mlow": c(mlow, np.float32),
                "mup": c(mup, np.float32),
            }
        )
    return in_maps


_NC_CACHE = {}


def _get_nc(nchunk):
    if nchunk not in _NC_CACHE:
        _NC_CACHE[nchunk] = build_nc(nchunk)
    return _NC_CACHE[nchunk]


def kernel(**inputs) -> np.ndarray:
    nchunk = S_FULL // C
    nc = _get_nc(nchunk)
    in_maps = make_host_inputs(inputs, nchunk)
    res = run_bass_kernel_spmd(nc, in_maps, core_ids=list(range(8)))
    S = nchunk * C
    out = np.zeros((B, S, HID), np.float32)
    for core in range(8):
        out[core // 4] += np.asarray(res.results[core]["out"], np.float32)
    return out


# revision 45
# speedup vs baseline: 1.2931x; 1.1022x over previous
"""DeltaNet Trainium2 kernel — 8-core SPMD, one (batch, head) pair per core.

v2: bf16 datapath (fp32 PSUM accumulation), chunked delta rule (C=128) with a
per-chunk UT-transform matrix T = (I+X)(I+X^2)(I+X^4) precomputed off the
serial state chain; beta / l2-norm row sums as N=1 transposed matmuls; k and q
planes interleaved per d-tile so K*K^T and K*Q^T come out of one matmul pair;
RMS-norm sums via matmul on the transposed o with the rstd*alpha_q scaling
folded into the o_proj PSUM-drain copies.  Host folds norm_w into Wo, sums the
4 per-head partial o_proj outputs per batch.
"""

import os
import sys
from contextlib import ExitStack

import ml_dtypes
import numpy as np

for _p in ("/opt/trn_rl_repo", "/root/.axon_site/_ro/trn_rl_repo"):
    if os.path.isdir(_p) and _p not in sys.path:
        sys.path.insert(0, _p)

import concourse.bass as bass  # noqa: E402
import concourse.tile as tile  # noqa: E402
from concourse import bacc, mybir  # noqa: E402
from concourse.bass_utils import run_bass_kernel_spmd  # noqa: E402

F32 = mybir.dt.float32
F32R = mybir.dt.float32r
BF16 = mybir.dt.bfloat16
AF = mybir.ActivationFunctionType
OP = mybir.AluOpType

HID = 1024
D = 256
C = 128
KT = HID // 128
NH = 4
B = 2
S_FULL = 2048
LOOKAHEAD = 2


def build_nc(nchunk=S_FULL // C, dbg=False):
    S = nchunk * C
    scs = 512 if S >= 512 else S
    nsc = S // scs
    nc = bacc.Bacc("TRN2", target_bir_lowering=False, debug=False)

    xt_d = nc.dram_tensor("xt", [HID, S], BF16, kind="ExternalInput")
    wq_d = nc.dram_tensor("wq", [HID, D], BF16, kind="ExternalInput")
    wk_d = nc.dram_tensor("wk", [HID, D], BF16, kind="ExternalInput")
    wv_d = nc.dram_tensor("wv", [HID, D], BF16, kind="ExternalInput")
    wb_d = nc.dram_tensor("wb", [HID, 1], BF16, kind="ExternalInput")
    wo_d = nc.dram_tensor("wo", [D, HID], BF16, kind="ExternalInput")
    cdq_d = nc.dram_tensor("cdq", [128, 8 * 128], BF16, kind="ExternalInput")
    cdk_d = nc.dram_tensor("cdk", [128, 8 * 128], BF16, kind="ExternalInput")
    cdv_d = nc.dram_tensor("cdv", [128, 8 * 128], BF16, kind="ExternalInput")
    identb_d = nc.dram_tensor("identb", [128, 128], BF16, kind="ExternalInput")
    identf_d = nc.dram_tensor("identf", [128, 128], F32R, kind="ExternalInput")
    identb2_d = nc.dram_tensor("identb2", [128, 256], BF16, kind="ExternalInput")
    onescol_d = nc.dram_tensor("onescol", [128, 1], BF16, kind="ExternalInput")
    mlow_d = nc.dram_tensor("mlow", [128, 128], F32, kind="ExternalInput")
    mup_d = nc.dram_tensor("mup", [128, 128], F32, kind="ExternalInput")
    out_d = nc.dram_tensor("out", [S, HID], F32, kind="ExternalOutput")
    dbg_d = {}
    if dbg:
        for nm, w_ in (("kq0", 2 * S), ("kq1", 2 * S), ("v0", S), ("v1", S),
                       ("tok", 4 * nchunk), ("X", 128 * nchunk),
                       ("T", 128 * nchunk), ("u", 256 * nchunk),
                       ("Gt", 256 * nchunk), ("ktok", 256 * nchunk),
                       ("w", 256 * nchunk), ("o_sb", 256 * nchunk),
                       ("S_sb", 512 * nchunk)):
            dt_ = F32 if nm == "tok" else BF16
            dbg_d[nm] = nc.dram_tensor(f"dbg_{nm}", [128, w_], dt_,
                                       kind="ExternalOutput")

    with tile.TileContext(nc) as tc, ExitStack() as ctx:
        # ---------------- persistent pools ----------------
        pconst = ctx.enter_context(tc.tile_pool(name="pconst", bufs=1))
        pplane = ctx.enter_context(tc.tile_pool(name="pplane", bufs=1))
        pw = ctx.enter_context(tc.tile_pool(name="pw", bufs=1))
        pxt = ctx.enter_context(tc.tile_pool(name="pxt", bufs=1))

        identb = pconst.tile([128, 128], BF16)
        identb2 = pconst.tile([128, 256], BF16)
        identf = pconst.tile([128, 128], F32R)
        onescol = pconst.tile([128, 1], BF16)
        rows4 = pconst.tile([4, S], F32R)  # 0=bk, 1=nbk2, 2=aq, 3=aq2
        mlow = pconst.tile([128, 128], F32)
        mup = pconst.tile([128, 128], F32)
        eps6 = pconst.tile([128, 1], F32)
        eps5 = pconst.tile([128, 1], F32)
        nc.vector.memset(eps6, 1e-6)
        nc.vector.memset(eps5, 1e-5)

        wo_sb = pw.tile([128, 2, HID], BF16)
        wb_sb = pw.tile([128, KT, 1], BF16)

        # planes: kq{dt} holds k in [:,0,:] and q in [:,1,:]; v separate
        kq0 = pplane.tile([128, 2, S], BF16, name="kq0")
        kq1 = pplane.tile([128, 2, S], BF16, name="kq1")
        v0 = pplane.tile([128, S], BF16, name="v0")
        v1 = pplane.tile([128, S], BF16, name="v1")

        xt_sb = pxt.tile([128, KT, S], BF16)

        # ---------------- phase B: projections + conv + silu ----------------
        with ExitStack() as bctx:
            pwt = bctx.enter_context(tc.tile_pool(name="pwt", bufs=2))
            pdiag = bctx.enter_context(tc.tile_pool(name="pdiag", bufs=2))
            praw = bctx.enter_context(tc.tile_pool(name="praw", bufs=2))
            ppb = bctx.enter_context(tc.tile_pool(name="ppb", bufs=5, space="PSUM"))

            nc.sync.dma_start(
                out=wb_sb, in_=wb_d.ap().rearrange("(k p) o -> p k o", p=128)
            )
            wd_srcs = {"q": wq_d, "k": wk_d, "v": wv_d}
            w_sbs = {}
            w_sbs["k"] = pwt.tile([128, KT, D], BF16, tag="w", name="w_k", bufs=3)
            nc.sync.dma_start(
                out=w_sbs["k"],
                in_=wd_srcs["k"].ap().rearrange("(k p) d -> p k d", p=128),
            )
            xt_src = xt_d.ap().rearrange("(k p) s -> p k s", p=128)
            for kk in range(KT):
                nc.sync.dma_start(out=xt_sb[:, kk, :], in_=xt_src[:, kk, :])

            copy_flip = 0
            for t in ("k", "q", "v"):
                cd_d = {"q": cdq_d, "k": cdk_d, "v": cdv_d}[t]
                if t not in w_sbs:
                    w_sbs[t] = pwt.tile(
                        [128, KT, D], BF16, tag="w", name=f"w_{t}", bufs=3
                    )
                    nc.sync.dma_start(
                        out=w_sbs[t],
                        in_=wd_srcs[t].ap().rearrange("(k p) d -> p k d", p=128),
                    )
                w_sb = w_sbs[t]
                diag = pdiag.tile([128, 8 * 128], BF16, tag="diag", name=f"diag_{t}")
                nc.sync.dma_start(out=diag, in_=cd_d.ap())
                for dt_ in range(2):
                    raw = praw.tile(
                        [128, S + 8], BF16, tag="raw", name=f"raw_{t}{dt_}"
                    )
                    nc.gpsimd.memset(raw[:, 0:8], 0.0)
                    for sc in range(nsc):
                        ps = ppb.tile([128, scs], F32, tag="ps", name="psraw")
                        for kk in range(KT):
                            nc.tensor.matmul(
                                ps,
                                w_sb[:, kk, dt_ * 128 : (dt_ + 1) * 128],
                                xt_sb[:, kk, sc * scs : (sc + 1) * scs],
                                start=(kk == 0),
                                stop=(kk == KT - 1),
                            )
                        dst = raw[:, 8 + sc * scs : 8 + (sc + 1) * scs]
                        if copy_flip % 2 == 0:
                            nc.vector.tensor_copy(dst, ps)
                        else:
                            nc.scalar.activation(out=dst, in_=ps, func=AF.Copy)
                        copy_flip += 1
                    # conv (4 taps as diagonal-stationary matmuls) + SiLU
                    if t == "v":
                        pdst = (v0, v1)[dt_]
                        dsts = [
                            pdst[:, sc * scs : (sc + 1) * scs] for sc in range(nsc)
                        ]
                    else:
                        kqp = (kq0, kq1)[dt_]
                        ti = 0 if t == "k" else 1
                        dsts = [
                            kqp[:, ti, sc * scs : (sc + 1) * scs]
                            for sc in range(nsc)
                        ]
                    for sc in range(nsc):
                        base = sc * scs
                        psc = ppb.tile([128, scs], F32, tag="ps", name="psconv")
                        for j in (3, 2, 1, 0):
                            sh = 3 - j
                            dslc = diag[
                                :, (j * 2 + dt_) * 128 : (j * 2 + dt_ + 1) * 128
                            ]
                            nc.tensor.matmul(
                                psc,
                                dslc,
                                raw[:, 8 + base - sh : 8 + base + scs - sh],
                                start=(j == 3),
                                stop=(j == 0),
                            )
                        nc.scalar.activation(out=dsts[sc], in_=psc, func=AF.Silu)

            # deferred small DMAs
            nc.sync.dma_start(out=identb, in_=identb_d.ap())
            nc.sync.dma_start(out=identb2, in_=identb2_d.ap())
            nc.sync.dma_start(out=identf, in_=identf_d.ap())
            nc.sync.dma_start(out=onescol, in_=onescol_d.ap())
            nc.sync.dma_start(out=mlow, in_=mlow_d.ap())
            nc.sync.dma_start(out=mup, in_=mup_d.ap())
            nc.sync.dma_start(
                out=wo_sb, in_=wo_d.ap().rearrange("(t p) h -> p t h", p=128)
            )

        # ---------------- phase C: token-scalar rows ----------------
        # All transcendentals happen here at row granularity so the Act
        # engine stays on one function-table set through the chunk loop
        # (Copy/Square/Sqrt share every set; Sigmoid/Silu force swaps).
        with ExitStack() as cctx:
            pcsq = cctx.enter_context(tc.tile_pool(name="pcsq", bufs=3))
            pcrow = cctx.enter_context(tc.tile_pool(name="pcrow", bufs=3))
            ppc = cctx.enter_context(tc.tile_pool(name="ppc", bufs=4, space="PSUM"))

            beta_row = pcrow.tile([1, S], F32, tag="beta", name="beta_row", bufs=1)
            ak_row = pcrow.tile([1, S], F32, tag="ak", name="ak_row", bufs=1)
            bk_row = pcrow.tile([1, S], F32R, tag="bk", name="bk_row", bufs=1)
            nbk2_row = pcrow.tile([1, S], F32R, tag="nbk2", name="nbk2_row", bufs=1)
            aq_row = pcrow.tile([1, S], F32R, tag="aq", name="aq_row", bufs=1)
            aq2_row = pcrow.tile([1, S], F32R, tag="aq2", name="aq2_row", bufs=1)
            for sc in range(nsc):
                sl = slice(sc * scs, (sc + 1) * scs)
                psb = ppc.tile([1, scs], F32, tag="ps", name="psb")
                for kk in range(KT):
                    nc.tensor.matmul(
                        psb, wb_sb[:, kk, :], xt_sb[:, kk, sl],
                        start=(kk == 0), stop=(kk == KT - 1),
                    )
                nc.scalar.activation(out=beta_row[0:1, sl], in_=psb, func=AF.Sigmoid)
            for t in ("k", "q"):
                kqp0 = kq0[:, 0 if t == "k" else 1, :]
                kqp1 = kq1[:, 0 if t == "k" else 1, :]
                for sc in range(nsc):
                    sl = slice(sc * scs, (sc + 1) * scs)
                    psl = ppc.tile([1, scs], F32, tag="ps", name="psl")
                    for dt_, pl in ((0, kqp0), (1, kqp1)):
                        sq = pcsq.tile([128, scs], BF16, tag="sq", name="sq")
                        nc.vector.tensor_mul(sq, pl[:, sl], pl[:, sl])
                        nc.tensor.matmul(
                            psl, onescol, sq, start=(dt_ == 0), stop=(dt_ == 1)
                        )
                    srow = pcrow.tile([1, scs], F32, tag="srow", name="srow")
                    nc.scalar.activation(
                        out=srow, in_=psl, func=AF.Sqrt, bias=eps6[0:1, :]
                    )
                    with nc.allow_low_precision(reason="f32r tok rows"):
                        if t == "k":
                            nc.vector.reciprocal(out=ak_row[0:1, sl], in_=srow)
                            nc.vector.tensor_mul(
                                bk_row[0:1, sl], beta_row[0:1, sl], ak_row[0:1, sl]
                            )
                            nc.vector.scalar_tensor_tensor(
                                out=nbk2_row[0:1, sl], in0=bk_row[0:1, sl],
                                scalar=-1.0, in1=ak_row[0:1, sl],
                                op0=OP.mult, op1=OP.mult,
                            )
                            nc.sync.dma_start(
                                out=rows4[0:1, sl], in_=bk_row[0:1, sl])
                            nc.sync.dma_start(
                                out=rows4[1:2, sl], in_=nbk2_row[0:1, sl])
                        else:
                            nc.vector.reciprocal(out=aq_row[0:1, sl], in_=srow)
                            nc.vector.scalar_tensor_tensor(
                                out=aq2_row[0:1, sl], in0=aq_row[0:1, sl],
                                scalar=1.0 / D, in1=aq_row[0:1, sl],
                                op0=OP.mult, op1=OP.mult,
                            )
                            nc.sync.dma_start(
                                out=rows4[2:3, sl], in_=aq_row[0:1, sl])
                            nc.sync.dma_start(
                                out=rows4[3:4, sl], in_=aq2_row[0:1, sl])

        # ---------------- phase D: chunked delta rule ----------------
        dctx = ExitStack()
        WIN = LOOKAHEAD + 2
        pS = dctx.enter_context(tc.tile_pool(name="pS", bufs=2))
        pcs = dctx.enter_context(tc.tile_pool(name="pcs", bufs=2))
        pcm = dctx.enter_context(tc.tile_pool(name="pcm", bufs=2))
        pwin = dctx.enter_context(tc.tile_pool(name="pwin", bufs=WIN))
        ptok = dctx.enter_context(tc.tile_pool(name="ptok", bufs=nchunk))
        pout = dctx.enter_context(tc.tile_pool(name="pout", bufs=2))
        ppS = dctx.enter_context(tc.tile_pool(name="ppS", bufs=1, space="PSUM"))
        ppw = dctx.enter_context(tc.tile_pool(name="ppw", bufs=1, space="PSUM"))
        ppo = dctx.enter_context(tc.tile_pool(name="ppo", bufs=2, space="PSUM"))
        ppt = dctx.enter_context(tc.tile_pool(name="ppt", bufs=3, space="PSUM"))

        psS0 = ppS.tile([128, 256], F32, tag="psS0", name="psS0")
        psS1 = ppS.tile([128, 256], F32, tag="psS1", name="psS1")

        state = {}

        def emit_pass1_pair(pi):
            """Chunk pair (2*pi, 2*pi+1): the Neumann ladder products are all
            single-shot matmuls, so both chunks share [128,256] pair tiles —
            one PSUM-drain copy advances both ladders."""
            ca, cb = 2 * pi, 2 * pi + 1
            chs = [slice(ca * C, (ca + 1) * C), slice(cb * C, (cb + 1) * C)]

            # token scalars for both chunks in one transpose+copy
            psR = ppt.tile([128, 8], F32R, tag="ps", name="psR")
            nc.tensor.transpose(psR[:, 0:4], rows4[0:4, chs[0]], identf[0:4, 0:4])
            nc.tensor.transpose(psR[:, 4:8], rows4[0:4, chs[1]], identf[0:4, 0:4])
            tokp = ptok.tile([128, 8], F32, tag="tok", name="tok")
            nc.vector.tensor_copy(tokp, psR)

            psAH = []
            for cc in range(2):
                p = ppt.tile([128, 256], F32, tag="ps", name=f"psAH{cc}")
                nc.tensor.matmul(p, kq0[:, 0, chs[cc]], kq0[:, :, chs[cc]],
                                 start=True, stop=False)
                nc.tensor.matmul(p, kq1[:, 0, chs[cc]], kq1[:, :, chs[cc]],
                                 start=False, stop=True)
                psAH.append(p)
            X = pcs.tile([128, 256], BF16, tag="X", name="X", bufs=2)
            Hm = pwin.tile([128, 256], BF16, tag="Hm", name="Hm")
            for cc in range(2):
                h = slice(cc * 128, (cc + 1) * 128)
                nc.vector.scalar_tensor_tensor(
                    out=X[:, h], in0=psAH[cc][:, 0:128],
                    scalar=tokp[:, cc * 4 + 1 : cc * 4 + 2], in1=mlow,
                    op0=OP.mult, op1=OP.mult,
                )
                nc.vector.tensor_mul(Hm[:, h], psAH[cc][:, 128:256], mup)

            def pairmm(lhsP, rhsP, name):
                ps = ppt.tile([128, 256], F32, tag="ps", name=name)
                for cc in range(2):
                    h = slice(cc * 128, (cc + 1) * 128)
                    nc.tensor.matmul(ps[:, h], lhsP[:, h], rhsP[:, h],
                                     start=True, stop=True)
                return ps

            psZ = ppt.tile([128, 256], BF16, tag="ps", name="psZ")
            nc.tensor.transpose(psZ[:, 0:128], X[:, 0:128], identb)
            nc.tensor.transpose(psZ[:, 128:256], X[:, 128:256], identb)
            Z = pcs.tile([128, 256], BF16, tag="Z", name="Z", bufs=2)
            nc.scalar.activation(out=Z, in_=psZ, func=AF.Copy)
            ZI = pcs.tile([128, 256], BF16, tag="ZI", name="ZI", bufs=2)
            nc.vector.tensor_add(ZI, psZ, identb2)

            psX2 = pairmm(Z, X, "psX2")
            X2 = pcs.tile([128, 256], BF16, tag="X2", name="X2", bufs=2)
            nc.vector.tensor_copy(X2, psX2)
            X2I = pcs.tile([128, 256], BF16, tag="X2I", name="X2I", bufs=2)
            nc.vector.tensor_add(X2I, psX2, identb2)

            psZ2 = pairmm(X, Z, "psZ2")
            Z2 = pcs.tile([128, 256], BF16, tag="Z2", name="Z2", bufs=2)
            nc.scalar.activation(out=Z2, in_=psZ2, func=AF.Copy)

            psX4 = pairmm(Z2, X2, "psX4")
            X4I = pcs.tile([128, 256], BF16, tag="X4I", name="X4I", bufs=2)
            nc.vector.tensor_add(X4I, psX4, identb2)

            psXB = pairmm(ZI, X2I, "psXB")
            XB = pcs.tile([128, 256], BF16, tag="XB", name="XB", bufs=2)
            nc.scalar.activation(out=XB, in_=psXB, func=AF.Copy)
            psZB = ppt.tile([128, 256], BF16, tag="ps", name="psZB")
            nc.tensor.transpose(psZB[:, 0:128], XB[:, 0:128], identb)
            nc.tensor.transpose(psZB[:, 128:256], XB[:, 128:256], identb)
            ZB = pcs.tile([128, 256], BF16, tag="ZB", name="ZB", bufs=2)
            nc.vector.tensor_copy(ZB, psZB)

            # Tt = T^T = (I+Z4)(I+Z2)(I+Z) = X4I^T @ ZB per half; lhsT roles
            # below then give u = Tt^T vb = T vb and Gt = Kb^T Tt = (T Kb)^T.
            psT = pairmm(X4I, ZB, "psT")
            T = pcs.tile([128, 256], BF16, tag="T", name="T", bufs=2)
            nc.scalar.activation(out=T, in_=psT, func=AF.Copy)

            # v / k token-layout + vb/ktok/Kb per chunk
            vbs, ktoks, Kbs = [], [], []
            for cc in range(2):
                ch = chs[cc]
                psVK = ppt.tile([128, 512], BF16, tag="ps", name=f"psVK{cc}")
                nc.tensor.transpose(psVK[:, 0:128], v0[:, ch], identb)
                nc.tensor.transpose(psVK[:, 128:256], v1[:, ch], identb)
                nc.tensor.transpose(psVK[:, 256:384], kq0[:, 0, ch], identb)
                nc.tensor.transpose(psVK[:, 384:512], kq1[:, 0, ch], identb)
                vb = pcm.tile([128, 256], BF16, tag="vb", name="vb", bufs=3)
                nc.vector.tensor_scalar(
                    out=vb, in0=psVK[:, 0:256],
                    scalar1=tokp[:, cc * 4 : cc * 4 + 1], scalar2=None,
                    op0=OP.mult,
                )
                ktok = pwin.tile([128, 256], BF16, tag="ktok", name="ktok",
                                 bufs=2 * WIN)
                nc.scalar.activation(out=ktok, in_=psVK[:, 256:512], func=AF.Copy)
                vbs.append(vb)
                ktoks.append(ktok)
                if 2 * pi + cc > 0:
                    Kb = pcm.tile([128, 256], BF16, tag="Kb", name="Kb", bufs=3)
                    nc.vector.tensor_scalar(
                        out=Kb, in0=psVK[:, 256:512],
                        scalar1=tokp[:, cc * 4 + 1 : cc * 4 + 2], scalar2=None,
                        op0=OP.mult,
                    )
                    Kbs.append(Kb)
                else:
                    Kbs.append(None)

            # u pair: [tok, v] halves for both chunks in one bank
            psU = ppt.tile([128, 512], F32, tag="ps", name="psU")
            nc.tensor.matmul(psU[:, 0:256], T[:, 0:128], vbs[0],
                             start=True, stop=True)
            nc.tensor.matmul(psU[:, 256:512], T[:, 128:256], vbs[1],
                             start=True, stop=True)
            u = pwin.tile([128, 512], BF16, tag="u", name="u")
            nc.scalar.activation(out=u, in_=psU, func=AF.Copy)

            # Gt pair: [d, tok] halves (4 single-shot mms, one copy)
            psGt = ppt.tile([128, 512], F32, tag="ps", name="psGt")
            for cc in range(2):
                if Kbs[cc] is None:
                    continue
                for dh in range(2):
                    nc.tensor.matmul(
                        psGt[:, cc * 256 + dh * 128 : cc * 256 + (dh + 1) * 128],
                        Kbs[cc][:, dh * 128 : (dh + 1) * 128],
                        T[:, cc * 128 : (cc + 1) * 128],
                        start=True, stop=True,
                    )
            Gt = pwin.tile([128, 512], BF16, tag="Gt", name="Gt")
            if pi == 0:
                nc.scalar.activation(out=Gt[:, 256:512], in_=psGt[:, 256:512],
                                     func=AF.Copy)
            else:
                nc.scalar.activation(out=Gt, in_=psGt, func=AF.Copy)

            for cc in range(2):
                i = 2 * pi + cc
                state[i] = dict(
                    aq_t=tokp[:, cc * 4 + 2 : cc * 4 + 3],
                    aq2_t=tokp[:, cc * 4 + 3 : cc * 4 + 4],
                    tokp=tokp,
                    Hm=Hm[:, cc * 128 : (cc + 1) * 128],
                    ktok=ktoks[cc],
                    u=u[:, cc * 256 : (cc + 1) * 256],
                    Gt0=Gt[:, cc * 256 : cc * 256 + 128],
                    Gt1=Gt[:, cc * 256 + 128 : (cc + 1) * 256],
                )
            if dbg:
                for cc in range(2):
                    i = 2 * pi + cc
                    nc.gpsimd.dma_start(
                        out=dbg_d["tok"].ap()[:, i*4:(i+1)*4],
                        in_=tokp[:, cc*4:(cc+1)*4])
                    nc.gpsimd.dma_start(
                        out=dbg_d["X"].ap()[:, i*128:(i+1)*128],
                        in_=X[:, cc*128:(cc+1)*128])
                    nc.gpsimd.dma_start(
                        out=dbg_d["T"].ap()[:, i*128:(i+1)*128],
                        in_=T[:, cc*128:(cc+1)*128])
                    nc.gpsimd.dma_start(
                        out=dbg_d["u"].ap()[:, i*256:(i+1)*256],
                        in_=u[:, cc*256:(cc+1)*256])
                    nc.gpsimd.dma_start(
                        out=dbg_d["ktok"].ap()[:, i*256:(i+1)*256],
                        in_=ktoks[cc])
                    if i > 0:
                        nc.gpsimd.dma_start(
                            out=dbg_d["Gt"].ap()[:, i*256:(i+1)*256],
                            in_=Gt[:, cc*256:(cc+1)*256])

        def emit_chain(i):
            ch = slice(i * C, (i + 1) * C)
            st = state[i]
            S_sb = None
            if i > 0:
                S_sb = pS.tile([128, 512], BF16, tag="S", name="S_sb")
                nc.scalar.activation(out=S_sb[:, 0:256], in_=psS0, func=AF.Copy)
                nc.vector.tensor_copy(S_sb[:, 256:512], psS1)

            if i > 0:
                psW = ppw.tile([128, 256], F32, tag="cw", name="psW")
                nc.tensor.matmul(
                    psW, st["Gt0"], S_sb[:, 0:256],
                    start=True, stop=False,
                )
                nc.tensor.matmul(
                    psW, st["Gt1"], S_sb[:, 256:512],
                    start=False, stop=True,
                )
                w = pcm.tile([128, 256], BF16, tag="w", name="w", bufs=3)
                nc.vector.tensor_add(w, psW, st["u"])
            else:
                w = st["u"]
            if dbg:
                nc.gpsimd.dma_start(out=dbg_d["w"].ap()[:, i*256:(i+1)*256], in_=w)
                if i > 0:
                    nc.gpsimd.dma_start(
                        out=dbg_d["S_sb"].ap()[:, i*512:(i+1)*512], in_=S_sb)

            # state update first so the chain keeps moving
            nc.tensor.matmul(
                psS0, st["ktok"][:, 0:128], w,
                start=(i == 0), stop=(i == nchunk - 1), skip_group_check=True,
            )
            nc.tensor.matmul(
                psS1, st["ktok"][:, 128:256], w,
                start=(i == 0), stop=(i == nchunk - 1), skip_group_check=True,
            )

            pso = ppo.tile([128, 256], F32, tag="po", name="pso")
            if i > 0:
                nc.tensor.matmul(pso, kq0[:, 1, ch], S_sb[:, 0:256],
                                 start=True, stop=False)
                nc.tensor.matmul(pso, kq1[:, 1, ch], S_sb[:, 256:512],
                                 start=False, stop=False)
                nc.tensor.matmul(pso, st["Hm"], w, start=False, stop=True)
            else:
                nc.tensor.matmul(pso, st["Hm"], w, start=True, stop=True)
            st["pso"] = pso

        def emit_trailing(i):
            ch = slice(i * C, (i + 1) * C)
            st = state.pop(i)
            pso = st["pso"]

            o_sb = pcm.tile([128, 256], BF16, tag="o_sb", name="o_sb", bufs=2)
            nc.vector.tensor_copy(o_sb, pso)
            if dbg:
                nc.gpsimd.dma_start(
                    out=dbg_d["o_sb"].ap()[:, i*256:(i+1)*256], in_=o_sb)
            psOT = ppt.tile([128, 256], BF16, tag="ps", name="psOT")
            nc.tensor.transpose(psOT[:, 0:128], o_sb[:, 0:128], identb)
            nc.tensor.transpose(psOT[:, 128:256], o_sb[:, 128:256], identb)
            ot = pcm.tile([128, 256], BF16, tag="ot", name="ot", bufs=2)
            nc.vector.tensor_copy(ot, psOT)

            # rms sums via Act square-with-accumulate (table-set resident);
            # reads o_sb (SBUF) so the pso PSUM bank frees after one copy
            sums = ptok.tile([128, 1], F32, tag="sums", name="sums")
            scratch = pcm.tile([128, 256], F32, tag="scr2", name="scratch", bufs=2)
            nc.scalar.activation(
                out=scratch, in_=o_sb, func=AF.Square, accum_out=sums
            )
            # rs_aq = aq/sqrt(aq^2/D * sums + eps5)
            rs = pcs.tile([128, 3], F32, tag="rs", name="rs", bufs=2)
            nc.scalar.activation(
                out=rs[:, 0:1], in_=sums, func=AF.Sqrt,
                scale=st["aq2_t"], bias=eps5,
            )
            nc.vector.reciprocal(out=rs[:, 2:3], in_=rs[:, 0:1])
            nc.vector.tensor_mul(rs[:, 1:2], rs[:, 2:3], st["aq_t"])

            outbuf = pout.tile([128, HID], F32, tag="outbuf", name="outbuf")
            for hc in range(2):
                psop = ppt.tile([128, 512], F32, tag="ps", name="psop")
                nc.tensor.matmul(
                    psop, ot[:, 0:128], wo_sb[:, 0, hc * 512 : (hc + 1) * 512],
                    start=True, stop=False,
                )
                nc.tensor.matmul(
                    psop, ot[:, 128:256], wo_sb[:, 1, hc * 512 : (hc + 1) * 512],
                    start=False, stop=True,
                )
                dst = outbuf[:, hc * 512 : (hc + 1) * 512]
                if hc == 0:
                    nc.vector.tensor_scalar(
                        out=dst, in0=psop, scalar1=rs[:, 1:2], scalar2=None,
                        op0=OP.mult,
                    )
                else:
                    nc.scalar.activation(
                        out=dst, in_=psop, func=AF.Copy, scale=rs[:, 1:2]
                    )
            nc.gpsimd.dma_start(out=out_d.ap()[ch, :], in_=outbuf)

        npair = nchunk // 2
        LAP = max(1, int(os.environ.get("KLOOKAHEAD", LOOKAHEAD)))
        for p in range(min(LAP, npair)):
            emit_pass1_pair(p)
        for i in range(nchunk):
            emit_chain(i)
            if i % 2 == 1:
                p = i // 2 + LAP
                if p < npair:
                    emit_pass1_pair(p)
            if i > 0:
                emit_trailing(i - 1)
        emit_trailing(nchunk - 1)
        if dbg:
            nc.gpsimd.dma_start(
                out=dbg_d["kq0"].ap(), in_=kq0.rearrange("p a b -> p (a b)"))
            nc.gpsimd.dma_start(
                out=dbg_d["kq1"].ap(), in_=kq1.rearrange("p a b -> p (a b)"))
            nc.gpsimd.dma_start(out=dbg_d["v0"].ap(), in_=v0)
            nc.gpsimd.dma_start(out=dbg_d["v1"].ap(), in_=v1)
        dctx.close()

    nc.compile()
    return nc


def make_host_inputs(inputs, nchunk=S_FULL // C):
    S = nchunk * C
    bf = ml_dtypes.bfloat16
    hs = np.asarray(inputs["hidden_states"], np.float32)[:, :S, :]
    Wq, Wk, Wv = (np.asarray(inputs[k], np.float32) for k in ("Wq", "Wk", "Wv"))
    Wb = np.asarray(inputs["Wb"], np.float32)
    Wo = np.asarray(inputs["Wo"], np.float32)
    nw = np.asarray(inputs["norm_w"], np.float32)
    convs = {
        k: np.asarray(inputs[k], np.float32) for k in ("conv_q", "conv_k", "conv_v")
    }

    identb = np.eye(128, dtype=np.float32)
    onescol = np.ones((128, 1), np.float32)
    mlow = np.tril(np.ones((128, 128), np.float32), -1)
    mup = np.triu(np.ones((128, 128), np.float32), 0)

    def diag_pack(cw):
        out = np.zeros((128, 8 * 128), np.float32)
        for j in range(4):
            for dt_ in range(2):
                blk = np.diag(cw[dt_ * 128 : (dt_ + 1) * 128, j])
                out[:, (j * 2 + dt_) * 128 : (j * 2 + dt_ + 1) * 128] = blk
        return out

    def c(a, dt=bf):
        return np.ascontiguousarray(a).astype(dt)

    in_maps = []
    for core in range(8):
        b, h = core // 4, core % 4
        hsel = slice(h * D, (h + 1) * D)
        in_maps.append(
            {
                "xt": c(hs[b].T),
                "wq": c(Wq[:, hsel]),
                "wk": c(Wk[:, hsel]),
                "wv": c(Wv[:, hsel]),
                "wb": c(Wb[:, h : h + 1]),
                "wo": c(nw[:, None] * Wo[hsel, :]),
                "cdq": c(diag_pack(convs["conv_q"][hsel])),
                "cdk": c(diag_pack(convs["conv_k"][hsel])),
                "cdv": c(diag_pack(convs["conv_v"][hsel])),
                "identb": c(identb),
                "identf": c(identb, np.float32),
                "identb2": c(np.concatenate([identb, identb], axis=1)),
                "onescol": c(onescol),
                "mlow": c(mlow, np.float32),
                "mup": c(mup, np.float32),
            }
        )
    return in_maps


_NC_CACHE = {}


def _get_nc(nchunk):
    if nchunk not in _NC_CACHE:
        _NC_CACHE[nchunk] = build_nc(nchunk)
    return _NC_CACHE[nchunk]


def kernel(**inputs) -> np.ndarray:
    nchunk = S_FULL // C
    nc = _get_nc(nchunk)
    in_maps = make_host_inputs(inputs, nchunk)
    res = run_bass_kernel_spmd(nc, in_maps, core_ids=list(range(8)))
    S = nchunk * C
    out = np.zeros((B, S, HID), np.float32)
    for core in range(8):
        out[core // 4] += np.asarray(res.results[core]["out"], np.float32)
    return out


# revision 47
# speedup vs baseline: 1.3480x; 1.0424x over previous
"""DeltaNet Trainium2 kernel — 8-core SPMD, one (batch, head) pair per core.

v2: bf16 datapath (fp32 PSUM accumulation), chunked delta rule (C=128) with a
per-chunk UT-transform matrix T = (I+X)(I+X^2)(I+X^4) precomputed off the
serial state chain; beta / l2-norm row sums as N=1 transposed matmuls; k and q
planes interleaved per d-tile so K*K^T and K*Q^T come out of one matmul pair;
RMS-norm sums via matmul on the transposed o with the rstd*alpha_q scaling
folded into the o_proj PSUM-drain copies.  Host folds norm_w into Wo, sums the
4 per-head partial o_proj outputs per batch.
"""

import os
import sys
from contextlib import ExitStack

import ml_dtypes
import numpy as np

for _p in ("/opt/trn_rl_repo", "/root/.axon_site/_ro/trn_rl_repo"):
    if os.path.isdir(_p) and _p not in sys.path:
        sys.path.insert(0, _p)

import concourse.bass as bass  # noqa: E402
import concourse.tile as tile  # noqa: E402
from concourse import bacc, mybir  # noqa: E402
from concourse.bass_utils import run_bass_kernel_spmd  # noqa: E402

F32 = mybir.dt.float32
F32R = mybir.dt.float32r
BF16 = mybir.dt.bfloat16
AF = mybir.ActivationFunctionType
OP = mybir.AluOpType

HID = 1024
D = 256
C = 128
KT = HID // 128
NH = 4
B = 2
S_FULL = 2048
LOOKAHEAD = 2


def build_nc(nchunk=S_FULL // C, dbg=False):
    S = nchunk * C
    scs = 512 if S >= 512 else S
    nsc = S // scs
    nc = bacc.Bacc("TRN2", target_bir_lowering=False, debug=False)

    xt_d = nc.dram_tensor("xt", [HID, S], BF16, kind="ExternalInput")
    wq_d = nc.dram_tensor("wq", [HID, D], BF16, kind="ExternalInput")
    wk_d = nc.dram_tensor("wk", [HID, D], BF16, kind="ExternalInput")
    wv_d = nc.dram_tensor("wv", [HID, D], BF16, kind="ExternalInput")
    wb_d = nc.dram_tensor("wb", [HID, 1], BF16, kind="ExternalInput")
    wo_d = nc.dram_tensor("wo", [D, HID], BF16, kind="ExternalInput")
    cdq_d = nc.dram_tensor("cdq", [128, 8 * 128], BF16, kind="ExternalInput")
    cdk_d = nc.dram_tensor("cdk", [128, 8 * 128], BF16, kind="ExternalInput")
    cdv_d = nc.dram_tensor("cdv", [128, 8 * 128], BF16, kind="ExternalInput")
    identb_d = nc.dram_tensor("identb", [128, 128], BF16, kind="ExternalInput")
    identf_d = nc.dram_tensor("identf", [128, 128], F32R, kind="ExternalInput")
    identb2_d = nc.dram_tensor("identb2", [128, 256], BF16, kind="ExternalInput")
    identb4_d = nc.dram_tensor("identb4", [128, 512], BF16, kind="ExternalInput")
    onescol_d = nc.dram_tensor("onescol", [128, 1], BF16, kind="ExternalInput")
    mlow_d = nc.dram_tensor("mlow", [128, 128], F32, kind="ExternalInput")
    mup_d = nc.dram_tensor("mup", [128, 128], F32, kind="ExternalInput")
    out_d = nc.dram_tensor("out", [S, HID], F32, kind="ExternalOutput")
    dbg_d = {}
    if dbg:
        for nm, w_ in (("kq0", 2 * S), ("kq1", 2 * S), ("v0", S), ("v1", S),
                       ("tok", 4 * nchunk), ("X", 128 * nchunk),
                       ("T", 128 * nchunk), ("u", 256 * nchunk),
                       ("Gt", 256 * nchunk), ("ktok", 256 * nchunk),
                       ("w", 256 * nchunk), ("o_sb", 256 * nchunk),
                       ("S_sb", 512 * nchunk)):
            dt_ = F32 if nm == "tok" else BF16
            dbg_d[nm] = nc.dram_tensor(f"dbg_{nm}", [128, w_], dt_,
                                       kind="ExternalOutput")

    with tile.TileContext(nc) as tc, ExitStack() as ctx:
        # ---------------- persistent pools ----------------
        pconst = ctx.enter_context(tc.tile_pool(name="pconst", bufs=1))
        pplane = ctx.enter_context(tc.tile_pool(name="pplane", bufs=1))
        pw = ctx.enter_context(tc.tile_pool(name="pw", bufs=1))
        pxt = ctx.enter_context(tc.tile_pool(name="pxt", bufs=1))

        identb = pconst.tile([128, 128], BF16)
        identb2 = pconst.tile([128, 256], BF16)
        identb4 = pconst.tile([128, 512], BF16)
        identf = pconst.tile([128, 128], F32R)
        onescol = pconst.tile([128, 1], BF16)
        rows4 = pconst.tile([4, S], F32R)  # 0=bk, 1=nbk2, 2=aq, 3=aq2
        mlow = pconst.tile([128, 128], F32)
        mup = pconst.tile([128, 128], F32)
        eps6 = pconst.tile([128, 1], F32)
        eps5 = pconst.tile([128, 1], F32)
        nc.vector.memset(eps6, 1e-6)
        nc.vector.memset(eps5, 1e-5)

        wo_sb = pw.tile([128, 2, HID], BF16)
        wb_sb = pw.tile([128, KT, 1], BF16)

        # planes: kq{dt} holds k in [:,0,:] and q in [:,1,:]; v separate
        kq0 = pplane.tile([128, 2, S], BF16, name="kq0")
        kq1 = pplane.tile([128, 2, S], BF16, name="kq1")
        v0 = pplane.tile([128, S], BF16, name="v0")
        v1 = pplane.tile([128, S], BF16, name="v1")

        xt_sb = pxt.tile([128, KT, S], BF16)

        # ---------------- phase B: projections + conv + silu ----------------
        with ExitStack() as bctx:
            pwt = bctx.enter_context(tc.tile_pool(name="pwt", bufs=2))
            pdiag = bctx.enter_context(tc.tile_pool(name="pdiag", bufs=2))
            praw = bctx.enter_context(tc.tile_pool(name="praw", bufs=2))
            ppb = bctx.enter_context(tc.tile_pool(name="ppb", bufs=5, space="PSUM"))

            nc.sync.dma_start(
                out=wb_sb, in_=wb_d.ap().rearrange("(k p) o -> p k o", p=128)
            )
            wd_srcs = {"q": wq_d, "k": wk_d, "v": wv_d}
            w_sbs = {}
            w_sbs["k"] = pwt.tile([128, KT, D], BF16, tag="w", name="w_k", bufs=3)
            nc.sync.dma_start(
                out=w_sbs["k"],
                in_=wd_srcs["k"].ap().rearrange("(k p) d -> p k d", p=128),
            )
            xt_src = xt_d.ap().rearrange("(k p) s -> p k s", p=128)
            for kk in range(KT):
                nc.sync.dma_start(out=xt_sb[:, kk, :], in_=xt_src[:, kk, :])

            copy_flip = 0
            for t in ("k", "q", "v"):
                cd_d = {"q": cdq_d, "k": cdk_d, "v": cdv_d}[t]
                if t not in w_sbs:
                    w_sbs[t] = pwt.tile(
                        [128, KT, D], BF16, tag="w", name=f"w_{t}", bufs=3
                    )
                    nc.sync.dma_start(
                        out=w_sbs[t],
                        in_=wd_srcs[t].ap().rearrange("(k p) d -> p k d", p=128),
                    )
                w_sb = w_sbs[t]
                diag = pdiag.tile([128, 8 * 128], BF16, tag="diag", name=f"diag_{t}")
                nc.sync.dma_start(out=diag, in_=cd_d.ap())
                for dt_ in range(2):
                    raw = praw.tile(
                        [128, S + 8], BF16, tag="raw", name=f"raw_{t}{dt_}"
                    )
                    nc.gpsimd.memset(raw[:, 0:8], 0.0)
                    for sc in range(nsc):
                        ps = ppb.tile([128, scs], F32, tag="ps", name="psraw")
                        for kk in range(KT):
                            nc.tensor.matmul(
                                ps,
                                w_sb[:, kk, dt_ * 128 : (dt_ + 1) * 128],
                                xt_sb[:, kk, sc * scs : (sc + 1) * scs],
                                start=(kk == 0),
                                stop=(kk == KT - 1),
                            )
                        dst = raw[:, 8 + sc * scs : 8 + (sc + 1) * scs]
                        if copy_flip % 2 == 0:
                            nc.vector.tensor_copy(dst, ps)
                        else:
                            nc.scalar.activation(out=dst, in_=ps, func=AF.Copy)
                        copy_flip += 1
                    # conv (4 taps as diagonal-stationary matmuls) + SiLU
                    if t == "v":
                        pdst = (v0, v1)[dt_]
                        dsts = [
                            pdst[:, sc * scs : (sc + 1) * scs] for sc in range(nsc)
                        ]
                    else:
                        kqp = (kq0, kq1)[dt_]
                        ti = 0 if t == "k" else 1
                        dsts = [
                            kqp[:, ti, sc * scs : (sc + 1) * scs]
                            for sc in range(nsc)
                        ]
                    for sc in range(nsc):
                        base = sc * scs
                        psc = ppb.tile([128, scs], F32, tag="ps", name="psconv")
                        for j in (3, 2, 1, 0):
                            sh = 3 - j
                            dslc = diag[
                                :, (j * 2 + dt_) * 128 : (j * 2 + dt_ + 1) * 128
                            ]
                            nc.tensor.matmul(
                                psc,
                                dslc,
                                raw[:, 8 + base - sh : 8 + base + scs - sh],
                                start=(j == 3),
                                stop=(j == 0),
                            )
                        nc.scalar.activation(out=dsts[sc], in_=psc, func=AF.Silu)

            # deferred small DMAs
            nc.sync.dma_start(out=identb, in_=identb_d.ap())
            nc.sync.dma_start(out=identb2, in_=identb2_d.ap())
            nc.sync.dma_start(out=identb4, in_=identb4_d.ap())
            nc.sync.dma_start(out=identf, in_=identf_d.ap())
            nc.sync.dma_start(out=onescol, in_=onescol_d.ap())
            nc.sync.dma_start(out=mlow, in_=mlow_d.ap())
            nc.sync.dma_start(out=mup, in_=mup_d.ap())
            nc.sync.dma_start(
                out=wo_sb, in_=wo_d.ap().rearrange("(t p) h -> p t h", p=128)
            )

        # ---------------- phase C: token-scalar rows ----------------
        # All transcendentals happen here at row granularity so the Act
        # engine stays on one function-table set through the chunk loop
        # (Copy/Square/Sqrt share every set; Sigmoid/Silu force swaps).
        with ExitStack() as cctx:
            pcsq = cctx.enter_context(tc.tile_pool(name="pcsq", bufs=3))
            pcrow = cctx.enter_context(tc.tile_pool(name="pcrow", bufs=3))
            ppc = cctx.enter_context(tc.tile_pool(name="ppc", bufs=4, space="PSUM"))

            beta_row = pcrow.tile([1, S], F32, tag="beta", name="beta_row", bufs=1)
            ak_row = pcrow.tile([1, S], F32, tag="ak", name="ak_row", bufs=1)
            bk_row = pcrow.tile([1, S], F32R, tag="bk", name="bk_row", bufs=1)
            nbk2_row = pcrow.tile([1, S], F32R, tag="nbk2", name="nbk2_row", bufs=1)
            aq_row = pcrow.tile([1, S], F32R, tag="aq", name="aq_row", bufs=1)
            aq2_row = pcrow.tile([1, S], F32R, tag="aq2", name="aq2_row", bufs=1)
            for sc in range(nsc):
                sl = slice(sc * scs, (sc + 1) * scs)
                psb = ppc.tile([1, scs], F32, tag="ps", name="psb")
                for kk in range(KT):
                    nc.tensor.matmul(
                        psb, wb_sb[:, kk, :], xt_sb[:, kk, sl],
                        start=(kk == 0), stop=(kk == KT - 1),
                    )
                nc.scalar.activation(out=beta_row[0:1, sl], in_=psb, func=AF.Sigmoid)
            for t in ("k", "q"):
                kqp0 = kq0[:, 0 if t == "k" else 1, :]
                kqp1 = kq1[:, 0 if t == "k" else 1, :]
                for sc in range(nsc):
                    sl = slice(sc * scs, (sc + 1) * scs)
                    psl = ppc.tile([1, scs], F32, tag="ps", name="psl")
                    for dt_, pl in ((0, kqp0), (1, kqp1)):
                        sq = pcsq.tile([128, scs], BF16, tag="sq", name="sq")
                        nc.vector.tensor_mul(sq, pl[:, sl], pl[:, sl])
                        nc.tensor.matmul(
                            psl, onescol, sq, start=(dt_ == 0), stop=(dt_ == 1)
                        )
                    srow = pcrow.tile([1, scs], F32, tag="srow", name="srow")
                    nc.scalar.activation(
                        out=srow, in_=psl, func=AF.Sqrt, bias=eps6[0:1, :]
                    )
                    with nc.allow_low_precision(reason="f32r tok rows"):
                        if t == "k":
                            nc.vector.reciprocal(out=ak_row[0:1, sl], in_=srow)
                            nc.vector.tensor_mul(
                                bk_row[0:1, sl], beta_row[0:1, sl], ak_row[0:1, sl]
                            )
                            nc.vector.scalar_tensor_tensor(
                                out=nbk2_row[0:1, sl], in0=bk_row[0:1, sl],
                                scalar=-1.0, in1=ak_row[0:1, sl],
                                op0=OP.mult, op1=OP.mult,
                            )
                            nc.sync.dma_start(
                                out=rows4[0:1, sl], in_=bk_row[0:1, sl])
                            nc.sync.dma_start(
                                out=rows4[1:2, sl], in_=nbk2_row[0:1, sl])
                        else:
                            nc.vector.reciprocal(out=aq_row[0:1, sl], in_=srow)
                            nc.vector.scalar_tensor_tensor(
                                out=aq2_row[0:1, sl], in0=aq_row[0:1, sl],
                                scalar=1.0 / D, in1=aq_row[0:1, sl],
                                op0=OP.mult, op1=OP.mult,
                            )
                            nc.sync.dma_start(
                                out=rows4[2:3, sl], in_=aq_row[0:1, sl])
                            nc.sync.dma_start(
                                out=rows4[3:4, sl], in_=aq2_row[0:1, sl])

        # ---------------- phase D: chunked delta rule ----------------
        dctx = ExitStack()
        WIN = LOOKAHEAD + 2
        pS = dctx.enter_context(tc.tile_pool(name="pS", bufs=2))
        pcs = dctx.enter_context(tc.tile_pool(name="pcs", bufs=2))
        pcm = dctx.enter_context(tc.tile_pool(name="pcm", bufs=2))
        pwin = dctx.enter_context(tc.tile_pool(name="pwin", bufs=WIN))
        ptok = dctx.enter_context(tc.tile_pool(name="ptok", bufs=nchunk))
        pout = dctx.enter_context(tc.tile_pool(name="pout", bufs=2))
        ppS = dctx.enter_context(tc.tile_pool(name="ppS", bufs=1, space="PSUM"))
        ppw = dctx.enter_context(tc.tile_pool(name="ppw", bufs=1, space="PSUM"))
        ppo = dctx.enter_context(tc.tile_pool(name="ppo", bufs=2, space="PSUM"))
        ppt = dctx.enter_context(tc.tile_pool(name="ppt", bufs=3, space="PSUM"))

        psS0 = ppS.tile([128, 256], F32, tag="psS0", name="psS0")
        psS1 = ppS.tile([128, 256], F32, tag="psS1", name="psS1")

        state = {}

        def emit_pass1_quad(qi):
            """Chunks 4qi..4qi+3 share [128,512] quad tiles through the
            Neumann ladder — one PSUM-drain copy advances four ladders."""
            cis = [4 * qi + cc for cc in range(4)]
            chs = [slice(i * C, (i + 1) * C) for i in cis]

            psR = ppt.tile([128, 16], F32R, tag="ps", name="psR")
            for cc in range(4):
                nc.tensor.transpose(psR[:, 4 * cc : 4 * cc + 4],
                                    rows4[0:4, chs[cc]], identf[0:4, 0:4])
            tokq = ptok.tile([128, 16], F32, tag="tok", name="tok")
            nc.vector.tensor_copy(tokq, psR)

            psAH = []
            for cc in range(4):
                p = ppt.tile([128, 256], F32, tag="ps", name=f"psAH{cc}")
                nc.tensor.matmul(p, kq0[:, 0, chs[cc]], kq0[:, :, chs[cc]],
                                 start=True, stop=False)
                nc.tensor.matmul(p, kq1[:, 0, chs[cc]], kq1[:, :, chs[cc]],
                                 start=False, stop=True)
                psAH.append(p)
            X = pcs.tile([128, 512], BF16, tag="X", name="X", bufs=2)
            Hm = pwin.tile([128, 512], BF16, tag="Hm", name="Hm", bufs=3)
            for cc in range(4):
                h = slice(cc * 128, (cc + 1) * 128)
                nc.vector.scalar_tensor_tensor(
                    out=X[:, h], in0=psAH[cc][:, 0:128],
                    scalar=tokq[:, cc * 4 + 1 : cc * 4 + 2], in1=mlow,
                    op0=OP.mult, op1=OP.mult,
                )
                nc.vector.tensor_mul(Hm[:, h], psAH[cc][:, 128:256], mup)

            def quadmm(lhsP, rhsP, name):
                ps = ppt.tile([128, 512], F32, tag="ps", name=name)
                for cc in range(4):
                    h = slice(cc * 128, (cc + 1) * 128)
                    nc.tensor.matmul(ps[:, h], lhsP[:, h], rhsP[:, h],
                                     start=True, stop=True)
                return ps

            def quadtrans(src, name):
                ps = ppt.tile([128, 512], BF16, tag="ps", name=name)
                for cc in range(4):
                    h = slice(cc * 128, (cc + 1) * 128)
                    nc.tensor.transpose(ps[:, h], src[:, h], identb)
                return ps

            psZ = quadtrans(X, "psZ")
            Z = pcs.tile([128, 512], BF16, tag="Z", name="Z", bufs=2)
            nc.scalar.activation(out=Z, in_=psZ, func=AF.Copy)
            ZI = pcs.tile([128, 512], BF16, tag="ZI", name="ZI", bufs=2)
            nc.vector.tensor_add(ZI, psZ, identb4)

            psX2 = quadmm(Z, X, "psX2")
            X2 = pcs.tile([128, 512], BF16, tag="X2", name="X2", bufs=2)
            nc.vector.tensor_copy(X2, psX2)
            X2I = pcs.tile([128, 512], BF16, tag="X2I", name="X2I", bufs=2)
            nc.vector.tensor_add(X2I, psX2, identb4)

            psZ2 = quadmm(X, Z, "psZ2")
            Z2 = pcs.tile([128, 512], BF16, tag="Z2", name="Z2", bufs=2)
            nc.scalar.activation(out=Z2, in_=psZ2, func=AF.Copy)

            psX4 = quadmm(Z2, X2, "psX4")
            X4I = pcs.tile([128, 512], BF16, tag="X4I", name="X4I", bufs=2)
            nc.vector.tensor_add(X4I, psX4, identb4)

            psXB = quadmm(ZI, X2I, "psXB")
            XB = pcs.tile([128, 512], BF16, tag="XB", name="XB", bufs=2)
            nc.scalar.activation(out=XB, in_=psXB, func=AF.Copy)
            psZB = quadtrans(XB, "psZB")
            ZB = pcs.tile([128, 512], BF16, tag="ZB", name="ZB", bufs=2)
            nc.vector.tensor_copy(ZB, psZB)

            # Tt = T^T = (I+Z4)(I+Z2)(I+Z) = X4I^T @ ZB per chunk; lhsT roles
            # below give u = Tt^T vb = T vb and Gt = Kb^T Tt = (T Kb)^T.
            psT = quadmm(X4I, ZB, "psT")
            T = pcs.tile([128, 512], BF16, tag="T", name="T", bufs=2)
            nc.scalar.activation(out=T, in_=psT, func=AF.Copy)

            vbs, ktoks, Kbs = [], [], []
            for cc in range(4):
                ch = chs[cc]
                psVK = ppt.tile([128, 512], BF16, tag="ps", name=f"psVK{cc}")
                nc.tensor.transpose(psVK[:, 0:128], v0[:, ch], identb)
                nc.tensor.transpose(psVK[:, 128:256], v1[:, ch], identb)
                nc.tensor.transpose(psVK[:, 256:384], kq0[:, 0, ch], identb)
                nc.tensor.transpose(psVK[:, 384:512], kq1[:, 0, ch], identb)
                vb = pcm.tile([128, 256], BF16, tag="vb", name="vb", bufs=5)
                nc.vector.tensor_scalar(
                    out=vb, in0=psVK[:, 0:256],
                    scalar1=tokq[:, cc * 4 : cc * 4 + 1], scalar2=None,
                    op0=OP.mult,
                )
                ktok = pwin.tile([128, 256], BF16, tag="ktok", name="ktok",
                                 bufs=12)
                nc.scalar.activation(out=ktok, in_=psVK[:, 256:512],
                                     func=AF.Copy)
                vbs.append(vb)
                ktoks.append(ktok)
                if 4 * qi + cc > 0:
                    Kb = pcm.tile([128, 256], BF16, tag="Kb", name="Kb", bufs=5)
                    nc.vector.tensor_scalar(
                        out=Kb, in0=psVK[:, 256:512],
                        scalar1=tokq[:, cc * 4 + 1 : cc * 4 + 2], scalar2=None,
                        op0=OP.mult,
                    )
                    Kbs.append(Kb)
                else:
                    Kbs.append(None)

            us, Gts = [], []
            for half in range(2):
                psU = ppt.tile([128, 512], F32, tag="ps", name=f"psU{half}")
                for j in range(2):
                    cc = 2 * half + j
                    nc.tensor.matmul(
                        psU[:, j * 256 : (j + 1) * 256],
                        T[:, cc * 128 : (cc + 1) * 128], vbs[cc],
                        start=True, stop=True,
                    )
                u = pwin.tile([128, 512], BF16, tag="u", name="u", bufs=6)
                nc.scalar.activation(out=u, in_=psU, func=AF.Copy)
                us.append(u)

                psGt = ppt.tile([128, 512], F32, tag="ps", name=f"psGt{half}")
                wrote = False
                for j in range(2):
                    cc = 2 * half + j
                    if Kbs[cc] is None:
                        continue
                    wrote = True
                    for dh in range(2):
                        nc.tensor.matmul(
                            psGt[:, j * 256 + dh * 128 : j * 256 + (dh + 1) * 128],
                            Kbs[cc][:, dh * 128 : (dh + 1) * 128],
                            T[:, cc * 128 : (cc + 1) * 128],
                            start=True, stop=True,
                        )
                Gt = pwin.tile([128, 512], BF16, tag="Gt", name="Gt", bufs=6)
                if wrote:
                    if half == 0 and qi == 0:
                        nc.scalar.activation(out=Gt[:, 256:512],
                                             in_=psGt[:, 256:512], func=AF.Copy)
                    else:
                        nc.scalar.activation(out=Gt, in_=psGt, func=AF.Copy)
                Gts.append(Gt)

            for cc in range(4):
                i = cis[cc]
                half, j = cc // 2, cc % 2
                state[i] = dict(
                    aq_t=tokq[:, cc * 4 + 2 : cc * 4 + 3],
                    aq2_t=tokq[:, cc * 4 + 3 : cc * 4 + 4],
                    Hm=Hm[:, cc * 128 : (cc + 1) * 128],
                    ktok=ktoks[cc],
                    u=us[half][:, j * 256 : (j + 1) * 256],
                    Gt0=Gts[half][:, j * 256 : j * 256 + 128],
                    Gt1=Gts[half][:, j * 256 + 128 : (j + 1) * 256],
                )

        def emit_chain(i):
            ch = slice(i * C, (i + 1) * C)
            st = state[i]
            S_sb = None
            if i > 0:
                S_sb = pS.tile([128, 512], BF16, tag="S", name="S_sb")
                nc.scalar.activation(out=S_sb[:, 0:256], in_=psS0, func=AF.Copy)
                nc.vector.tensor_copy(S_sb[:, 256:512], psS1)

            if i > 0:
                psW = ppw.tile([128, 256], F32, tag="cw", name="psW")
                nc.tensor.matmul(
                    psW, st["Gt0"], S_sb[:, 0:256],
                    start=True, stop=False,
                )
                nc.tensor.matmul(
                    psW, st["Gt1"], S_sb[:, 256:512],
                    start=False, stop=True,
                )
                w = pcm.tile([128, 256], BF16, tag="w", name="w", bufs=3)
                nc.vector.tensor_add(w, psW, st["u"])
            else:
                w = st["u"]
            if dbg:
                nc.gpsimd.dma_start(out=dbg_d["w"].ap()[:, i*256:(i+1)*256], in_=w)
                if i > 0:
                    nc.gpsimd.dma_start(
                        out=dbg_d["S_sb"].ap()[:, i*512:(i+1)*512], in_=S_sb)

            # state update first so the chain keeps moving
            nc.tensor.matmul(
                psS0, st["ktok"][:, 0:128], w,
                start=(i == 0), stop=(i == nchunk - 1), skip_group_check=True,
            )
            nc.tensor.matmul(
                psS1, st["ktok"][:, 128:256], w,
                start=(i == 0), stop=(i == nchunk - 1), skip_group_check=True,
            )

            pso = ppo.tile([128, 256], F32, tag="po", name="pso")
            if i > 0:
                nc.tensor.matmul(pso, kq0[:, 1, ch], S_sb[:, 0:256],
                                 start=True, stop=False)
                nc.tensor.matmul(pso, kq1[:, 1, ch], S_sb[:, 256:512],
                                 start=False, stop=False)
                nc.tensor.matmul(pso, st["Hm"], w, start=False, stop=True)
            else:
                nc.tensor.matmul(pso, st["Hm"], w, start=True, stop=True)
            st["pso"] = pso

        def emit_trailing(i):
            ch = slice(i * C, (i + 1) * C)
            st = state.pop(i)
            pso = st["pso"]

            o_sb = pcm.tile([128, 256], BF16, tag="o_sb", name="o_sb", bufs=2)
            nc.vector.tensor_copy(o_sb, pso)
            if dbg:
                nc.gpsimd.dma_start(
                    out=dbg_d["o_sb"].ap()[:, i*256:(i+1)*256], in_=o_sb)
            psOT = ppt.tile([128, 256], BF16, tag="ps", name="psOT")
            nc.tensor.transpose(psOT[:, 0:128], o_sb[:, 0:128], identb)
            nc.tensor.transpose(psOT[:, 128:256], o_sb[:, 128:256], identb)
            ot = pcm.tile([128, 256], BF16, tag="ot", name="ot", bufs=2)
            nc.vector.tensor_copy(ot, psOT)

            # rms sums via Act square-with-accumulate (table-set resident);
            # reads o_sb (SBUF) so the pso PSUM bank frees after one copy
            sums = ptok.tile([128, 1], F32, tag="sums", name="sums")
            scratch = pcm.tile([128, 256], F32, tag="scr2", name="scratch", bufs=2)
            nc.scalar.activation(
                out=scratch, in_=o_sb, func=AF.Square, accum_out=sums
            )
            # rs_aq = aq/sqrt(aq^2/D * sums + eps5)
            rs = pcs.tile([128, 3], F32, tag="rs", name="rs", bufs=2)
            nc.scalar.activation(
                out=rs[:, 0:1], in_=sums, func=AF.Sqrt,
                scale=st["aq2_t"], bias=eps5,
            )
            nc.vector.reciprocal(out=rs[:, 2:3], in_=rs[:, 0:1])
            nc.vector.tensor_mul(rs[:, 1:2], rs[:, 2:3], st["aq_t"])

            outbuf = pout.tile([128, HID], F32, tag="outbuf", name="outbuf")
            for hc in range(2):
                psop = ppt.tile([128, 512], F32, tag="ps", name="psop")
                nc.tensor.matmul(
                    psop, ot[:, 0:128], wo_sb[:, 0, hc * 512 : (hc + 1) * 512],
                    start=True, stop=False,
                )
                nc.tensor.matmul(
                    psop, ot[:, 128:256], wo_sb[:, 1, hc * 512 : (hc + 1) * 512],
                    start=False, stop=True,
                )
                dst = outbuf[:, hc * 512 : (hc + 1) * 512]
                if hc == 0:
                    nc.vector.tensor_scalar(
                        out=dst, in0=psop, scalar1=rs[:, 1:2], scalar2=None,
                        op0=OP.mult,
                    )
                else:
                    nc.scalar.activation(
                        out=dst, in_=psop, func=AF.Copy, scale=rs[:, 1:2]
                    )
            nc.gpsimd.dma_start(out=out_d.ap()[ch, :], in_=outbuf)

        nquad = nchunk // 4
        LAQ = max(1, int(os.environ.get("KLOOKAHEAD", 2)))
        for q in range(min(LAQ, nquad)):
            emit_pass1_quad(q)
        for i in range(nchunk):
            emit_chain(i)
            if i % 4 == 3:
                q = i // 4 + LAQ
                if q < nquad:
                    emit_pass1_quad(q)
            if i > 0:
                emit_trailing(i - 1)
        emit_trailing(nchunk - 1)
        dctx.close()

    nc.compile()
    return nc


def make_host_inputs(inputs, nchunk=S_FULL // C):
    S = nchunk * C
    bf = ml_dtypes.bfloat16
    hs = np.asarray(inputs["hidden_states"], np.float32)[:, :S, :]
    Wq, Wk, Wv = (np.asarray(inputs[k], np.float32) for k in ("Wq", "Wk", "Wv"))
    Wb = np.asarray(inputs["Wb"], np.float32)
    Wo = np.asarray(inputs["Wo"], np.float32)
    nw = np.asarray(inputs["norm_w"], np.float32)
    convs = {
        k: np.asarray(inputs[k], np.float32) for k in ("conv_q", "conv_k", "conv_v")
    }

    identb = np.eye(128, dtype=np.float32)
    onescol = np.ones((128, 1), np.float32)
    mlow = np.tril(np.ones((128, 128), np.float32), -1)
    mup = np.triu(np.ones((128, 128), np.float32), 0)

    def diag_pack(cw):
        out = np.zeros((128, 8 * 128), np.float32)
        for j in range(4):
            for dt_ in range(2):
                blk = np.diag(cw[dt_ * 128 : (dt_ + 1) * 128, j])
                out[:, (j * 2 + dt_) * 128 : (j * 2 + dt_ + 1) * 128] = blk
        return out

    def c(a, dt=bf):
        return np.ascontiguousarray(a).astype(dt)

    in_maps = []
    for core in range(8):
        b, h = core // 4, core % 4
        hsel = slice(h * D, (h + 1) * D)
        in_maps.append(
            {
                "xt": c(hs[b].T),
                "wq": c(Wq[:, hsel]),
                "wk": c(Wk[:, hsel]),
                "wv": c(Wv[:, hsel]),
                "wb": c(Wb[:, h : h + 1]),
                "wo": c(nw[:, None] * Wo[hsel, :]),
                "cdq": c(diag_pack(convs["conv_q"][hsel])),
                "cdk": c(diag_pack(convs["conv_k"][hsel])),
                "cdv": c(diag_pack(convs["conv_v"][hsel])),
                "identb": c(identb),
                "identf": c(identb, np.float32),
                "identb2": c(np.concatenate([identb, identb], axis=1)),
                "identb4": c(np.concatenate([identb] * 4, axis=1)),
                "onescol": c(onescol),
                "mlow": c(mlow, np.float32),
                "mup": c(mup, np.float32),
            }
        )
    return in_maps


_NC_CACHE = {}


def _get_nc(nchunk):
    if nchunk not in _NC_CACHE:
        _NC_CACHE[nchunk] = build_nc(nchunk)
    return _NC_CACHE[nchunk]


def kernel(**inputs) -> np.ndarray:
    nchunk = S_FULL // C
    nc = _get_nc(nchunk)
    in_maps = make_host_inputs(inputs, nchunk)
    res = run_bass_kernel_spmd(nc, in_maps, core_ids=list(range(8)))
    S = nchunk * C
    out = np.zeros((B, S, HID), np.float32)
    for core in range(8):
        out[core // 4] += np.asarray(res.results[core]["out"], np.float32)
    return out
